# revision 14
# baseline (speedup 1.0000x reference)
"""Trainium2 Bass kernel for nn_Bert_10187662426159 (DeBERTa-style
disentangled-attention BERT layer, L=512 B=16 D=1024 H=16).

Sharding: data-parallel over B - core c handles batch entries {2c, 2c+1}.

Per-core pipeline (ST orientation: scores stored [key j on partitions,
query i on free dim]; matmul operands f16, PSUM accumulation f32):
  P1  LN1 (no affine) -> h ; PE-transpose -> hT [feat, tok].  The 63
      relative-embedding rows ride as 63 extra "tokens" (cols 1024:1087
      of hT), so the q/k projection also produces qpos/kpos for free.
  P1b q/k proj (feat-major, q half pre-scaled by 1/sqrt(3*64)) over the
      1088-token axis; v proj (token-major, with a ones-column per head
      for softmax row sums).
  P2  per (batch-entry, head), software-pipelined by one head:
        qp/pk rank-63 positional factors from qT/kT slices
        window expansion [128, 640] per 128-row tile via 0/1 G-matrices
        (raw scores in delta space, f32 PSUM -> f16 staging)
        skew via DRAM bounce: write staging [128, 2, 4, 640] contiguous,
        read back with flat row stride 639 elements (one diagonal-stride
        DMA realigns every row's shift 127-p exactly - same output as a
        per-partition shift)
      assembly per 128-row j-tile in one PSUM bank: c2c matmul + pk via
      identity-matmul + qp via four identity-rhs transpose-matmuls; ONE
      ACT exp with the attention mask as a per-partition bias (-1e9) ->
      P (f16; scores bounded, exp(-1e9) = 0, no max-subtraction)
      ctx: [v | 1]^T @ P accumulates context AND row sums in PSUM;
      1/sum broadcast via a rank-1 PE matmul; DVE multiply normalizes.
  P3  y = ctxT^T @ woT, LN2 + affine.

The DeBERTa take_along_axis gathers are exact: bucket expansion is a 0/1
matmul in delta-space and the diagonal re-alignment (skew) is an exact
strided DMA read from DRAM scratch (row stride 639 on a 640-wide array
walks one element left per partition row).
"""
import contextlib
import math
import sys

import numpy as np

sys.path.insert(0, "/opt/trn_rl_repo")
sys.path.insert(0, "/opt/trn_rl_repo/concourse")

import concourse.mybir as mybir  # noqa: E402
import concourse.tile as tile  # noqa: E402
from concourse import bacc, bass_utils  # noqa: E402
from concourse.ap import AP  # noqa: E402
from concourse.masks import make_identity  # noqa: E402

F32 = mybir.dt.float32
F16 = mybir.dt.float16
F8 = mybir.dt.float8e3

HIDDEN, HEADS, HEAD = 1024, 16, 64
BUCKET, MAXPOS, REL = 32, 512, 63
L, B = 512, 16
EPS = 1e-7
SCALE = 1.0 / math.sqrt(3 * HEAD)
WIN = 640
NCORES = 8
BLOC = B // NCORES          # 2 batch entries per core
NTOK = L * BLOC             # 1024 tokens per core
NT = NTOK // 128            # 8 token tiles
TOK2 = NTOK + 64            # tokens + 63 rel rows + 1 pad
AF = mybir.ActivationFunctionType


def _bucket_fn(delta):
    r = np.asarray(delta)
    mid = BUCKET // 2
    abs_pos = np.where((r < mid) & (r > -mid), mid - 1,
                       np.minimum(np.abs(r), MAXPOS - 1))
    with np.errstate(divide="ignore"):
        log_pos = (np.ceil(np.log(abs_pos.astype(np.float64) / mid)
                           / math.log((MAXPOS - 1) / mid) * (mid - 1))
                   .astype(np.int64) + mid)
    bucket_pos = np.where(abs_pos <= mid, r, log_pos * np.sign(r))
    return (BUCKET - 1 + bucket_pos).astype(np.int64)


def _make_G():
    Gq, Gk = [], []
    for t in range(4):
        w0 = -127 - 128 * t
        c = np.arange(WIN)
        dq = np.clip(-(w0 + c), -511, 511)
        dk = np.clip(+(w0 + c), -511, 511)
        Gq.append(_bucket_fn(dq)[None, :] == np.arange(REL)[:, None])
        Gk.append(_bucket_fn(dk)[None, :] == np.arange(REL)[:, None])
    f8 = mybir.dt.np(F8)
    return (np.stack(Gq).transpose(1, 0, 2).astype(f8),
            np.stack(Gk).transpose(1, 0, 2).astype(f8))  # [63, 4, 640]


def _build(with_bias: bool, with_affine: bool):
    nc = bacc.Bacc("TRN2", debug=False, num_devices=NCORES)

    hs_d = nc.dram_tensor("hs_tok", (NTOK, HIDDEN), F32, kind="ExternalInput").ap()
    mb_d = nc.dram_tensor("maskbias", (128, BLOC * 4), F32, kind="ExternalInput").ap()
    wqkT_d = nc.dram_tensor("wqkT", (16, 128, 8, 128), F16, kind="ExternalInput").ap()
    wvT_d = nc.dram_tensor("wvT", (HIDDEN, HIDDEN), F16, kind="ExternalInput").ap()
    woT_d = nc.dram_tensor("woT", (HIDDEN, HIDDEN), F16, kind="ExternalInput").ap()
    relT_d = nc.dram_tensor("relT", (HIDDEN, 64), F16, kind="ExternalInput").ap()
    gq_d = nc.dram_tensor("Gq", (REL, 4, WIN), F8, kind="ExternalInput").ap()
    gk_d = nc.dram_tensor("Gk", (REL, 4, WIN), F8, kind="ExternalInput").ap()
    if with_bias:
        bqk_d = nc.dram_tensor("bqk2", (1, 2 * HIDDEN), F16, kind="ExternalInput").ap()
        bv_d = nc.dram_tensor("bv2", (1, HIDDEN), F16, kind="ExternalInput").ap()
        ones_d = nc.dram_tensor("ones_row", (1, TOK2), F16, kind="ExternalInput").ap()
    if with_affine:
        g_d = nc.dram_tensor("g_bcast", (128, HIDDEN), F32, kind="ExternalInput").ap()
        b_d = nc.dram_tensor("b_bcast", (128, HIDDEN), F32, kind="ExternalInput").ap()
    out_d = nc.dram_tensor("out_y", (NTOK, HIDDEN), F32, kind="ExternalOutput").ap()
    # DRAM scratch for the skew bounce: one [128, 2, 4, 640] f16 block per
    # (batch-entry, head).
    scr_t = nc.dram_tensor("skew_scratch", (B // NCORES * HEADS, 128, 2, 4, WIN),
                           F8, kind="Internal")
    scr_d = scr_t.ap()
    SCR_BH = 128 * 2 * 4 * WIN  # elements per (bi, head) block

    with tile.TileContext(nc) as tc, contextlib.ExitStack() as ctx:
        consts = ctx.enter_context(tc.tile_pool(name="consts", bufs=1))
        wpool = ctx.enter_context(tc.tile_pool(name="wpool", bufs=3))
        xio = ctx.enter_context(tc.tile_pool(name="xio", bufs=2))
        stat = ctx.enter_context(tc.tile_pool(name="stat", bufs=4))
        big = ctx.enter_context(tc.tile_pool(name="big", bufs=1))
        stagp = ctx.enter_context(tc.tile_pool(name="stagp", bufs=2))
        winp = ctx.enter_context(tc.tile_pool(name="winp", bufs=3))
        attp = ctx.enter_context(tc.tile_pool(name="attp", bufs=2))
        # PSUM: psA 3x1 bank (scores/fac/rb), psE 2x2 (expansion),
        # psC 1x1 (ctx) -> 8 banks total.
        psA = ctx.enter_context(tc.tile_pool(name="psA", bufs=3, space="PSUM"))
        psE = ctx.enter_context(tc.tile_pool(name="psE", bufs=2, space="PSUM"))
        psC = ctx.enter_context(tc.tile_pool(name="psC", bufs=1, space="PSUM"))

        # ---------- constants ----------
        ident16 = consts.tile([128, 128], F16)
        make_identity(nc, ident16)
        ident8 = consts.tile([128, 128], F8)
        make_identity(nc, ident8)
        eps_t = consts.tile([128, 1], F32)
        nc.vector.memset(eps_t, EPS)
        one64 = consts.tile([1, 64], F16)
        nc.vector.memset(one64, 1.0)
        gq_s = consts.tile([REL, 4, WIN], F8)
        gk_s = consts.tile([REL, 4, WIN], F8)
        mb_s = consts.tile([128, BLOC * 4], F32)
        if with_bias:
            bqk_s = consts.tile([1, 2 * HIDDEN], F16)
            bv_s = consts.tile([1, HIDDEN], F16)
            ones_s = consts.tile([1, TOK2], F16)
            nc.sync.dma_start(out=bqk_s, in_=bqk_d)
            nc.sync.dma_start(out=bv_s, in_=bv_d)
            nc.sync.dma_start(out=ones_s, in_=ones_d)
        if with_affine:
            g_s = consts.tile([128, HIDDEN], F32)
            b_s = consts.tile([128, HIDDEN], F32)
            nc.sync.dma_start(out=g_s, in_=g_d)
            nc.sync.dma_start(out=b_s, in_=b_d)

        def layernorm_stats(y):
            """-> (rstd, -mean*rstd) [128,1] tiles for the normalize apply."""
            st = stat.tile([128, 2, nc.vector.BN_STATS_DIM], F32, tag="st")
            mv = stat.tile([128, nc.vector.BN_AGGR_DIM], F32, tag="mv")
            yr = y.rearrange("p (s d) -> p s d", s=2)
            for s in range(2):
                nc.vector.bn_stats(out=st[:, s, :], in_=yr[:, s, :])
            nc.vector.bn_aggr(out=mv, in_=st)
            rstd = stat.tile([128, 1], F32, tag="rstd")
            nc.scalar.activation(out=rstd, in_=mv[:, 1:2], func=AF.Sqrt,
                                 bias=eps_t, scale=1.0)
            nc.vector.reciprocal(out=rstd, in_=rstd)
            nmr = stat.tile([128, 1], F32, tag="nmr")
            nc.vector.tensor_mul(nmr, mv[:, 0:1], rstd)
            nc.vector.tensor_scalar_mul(nmr, nmr, -1.0)
            return rstd, nmr

        # ---------- P1: LN1 + transpose (rel rows ride as extra tokens) ----
        hT = big.tile([128, NT, TOK2], F16, tag="hT")  # [feat, tok]
        hs3 = hs_d.rearrange("(n p) d -> n p d", p=128)

        def emit_ln_tile(tt):
            x = xio.tile([128, HIDDEN], F32, tag="xy")
            nc.sync.dma_start(out=x, in_=hs3[tt])
            rstd, nmr = layernorm_stats(x)
            h = xio.tile([128, HIDDEN], F16, tag="hyo")
            nc.vector.tensor_scalar(out=h, in0=x, scalar1=rstd, scalar2=nmr,
                                    op0=mybir.AluOpType.mult,
                                    op1=mybir.AluOpType.add)
            for fb in range(NT):
                ptr = psA.tile([128, 128], F16, tag="b1")
                nc.tensor.matmul(ptr, h[:, 128 * fb:128 * fb + 128], ident16,
                                 is_transpose=True)
                nc.vector.tensor_copy(out=hT[:, fb, 128 * tt:128 * tt + 128], in_=ptr)

        for tt in range(4):
            emit_ln_tile(tt)
        nc.scalar.dma_start(out=hT[:, :, NTOK:NTOK + 64],
                            in_=relT_d.rearrange("(n p) r -> p n r", p=128))
        nc.vector.memset(hT[:, :, TOK2 - 1:TOK2], 0.0)
        nc.scalar.dma_start(out=gq_s, in_=gq_d)
        nc.scalar.dma_start(out=gk_s, in_=gk_d)
        nc.scalar.dma_start(out=mb_s, in_=mb_d)

        # ---------- P1b: projections ----------
        qT = big.tile([128, NT, TOK2], F16, tag="qT")
        kT = big.tile([128, NT, TOK2], F16, tag="kT")
        vtm = big.tile([128, NT, HEADS, HEAD + 1], F16, tag="v")
        nc.vector.memset(vtm[:, :, :, HEAD:HEAD + 1], 1.0)
        wqk3 = wqkT_d

        # q/k (+qpos/kpos): feat-major out [2048 -> 16 M-tiles, 1088 tok]
        facsAll = big.tile([REL, BLOC * HEADS, 2, 512], F8, tag="facs")

        def emit_factors(hd):
            pf = slice(64 * (hd % 2), 64 * (hd % 2) + 64)
            mgh = hd // 2
            for bi in range(BLOC):
                toks = slice(512 * bi, 512 * bi + 512)
                bh = HEADS * bi + hd
                fq_ps = psA.tile([REL, 512], F32, tag="b1")
                nc.tensor.matmul(fq_ps, kT[pf, mgh, NTOK:NTOK + REL],
                                 qT[pf, mgh, toks])
                fk_ps = psA.tile([REL, 512], F32, tag="b1")
                nc.tensor.matmul(fk_ps, qT[pf, mgh, NTOK:NTOK + REL],
                                 kT[pf, mgh, toks])
                if bh % 2 == 0:
                    nc.scalar.copy(out=facsAll[:, bh, 0, :], in_=fq_ps)
                    nc.vector.tensor_copy(out=facsAll[:, bh, 1, :], in_=fk_ps)
                else:
                    nc.vector.tensor_copy(out=facsAll[:, bh, 0, :], in_=fq_ps)
                    nc.scalar.copy(out=facsAll[:, bh, 1, :], in_=fk_ps)

        NCH = [(0, 512), (512, 512), (1024, 64)]

        def emit_qk_pass(chunks, second):
            for mg in range(16):
                w_m = wpool.tile([128, 8, 128], F16, tag="wqk")
                nc.scalar.dma_start(out=w_m, in_=wqk3[mg])
                for ci in chunks:
                    n0, nw = NCH[ci]
                    ns = slice(n0, n0 + nw)
                    pq = psA.tile([128, 512], F32, tag="b1")
                    for k in range(8):
                        nc.tensor.matmul(pq[:, :nw], w_m[:, k, :], hT[:, k, ns],
                                         start=(k == 0),
                                         stop=(k == 7 and not with_bias))
                    if with_bias:
                        nc.tensor.matmul(pq[:, :nw],
                                         bqk_s[:, 128 * mg:128 * mg + 128],
                                         ones_s[:, ns], start=False, stop=True)
                    dst = qT if mg < 8 else kT
                    eng = nc.vector if (mg * 3 + ci) % 2 == 0 else nc.scalar
                    if eng is nc.vector:
                        eng.tensor_copy(out=dst[:, mg % 8, ns], in_=pq[:, :nw])
                    else:
                        eng.copy(out=dst[:, mg % 8, ns], in_=pq[:, :nw])
                if second and mg >= 8:
                    emit_factors(2 * (mg - 8))
                    emit_factors(2 * (mg - 8) + 1)

        emit_qk_pass([0], False)        # tokens 0:512 (needs LN tiles 0-3)
        for tt in range(4, NT):
            emit_ln_tile(tt)
        emit_qk_pass([1, 2], True)      # tokens 512:1024 + rel, factors

        # v: token-major out [tok, feat]; wvT resident then woT reuses slot
        wv_s = big.tile([128, 8, HIDDEN], F16, tag="wvo")
        nc.sync.dma_start(out=wv_s, in_=wvT_d.rearrange("(n p) m -> p n m", p=128))
        for mt in range(NT):
            for nn_ in range(2):
                ns = slice(512 * nn_, 512 * nn_ + 512)
                pv = psA.tile([128, 512], F32, tag="b1")
                for k in range(8):
                    nc.tensor.matmul(pv, hT[:, k, 128 * mt:128 * mt + 128],
                                     wv_s[:, k, ns], start=(k == 0),
                                     stop=(k == 7 and not with_bias))
                if with_bias:
                    nc.tensor.matmul(pv, ones_s[:, 128 * mt:128 * mt + 128],
                                     bv_s[:, ns], start=False, stop=True)
                nc.vector.tensor_copy(
                    out=vtm[:, mt, 8 * nn_:8 * nn_ + 8, 0:HEAD],
                    in_=pv.rearrange("p (h d) -> p h d", d=HEAD))

        # wo load early (sync queue is idle; overlaps attention)
        wo_s = big.tile([128, 8, HIDDEN], F16, tag="wvo")  # reuse wv slot
        nc.sync.dma_start(out=wo_s, in_=woT_d.rearrange("(n p) m -> p n m", p=128))

        # ---------- P2: attention, software-pipelined by two heads ----------
        ctxT = big.tile([128, BLOC, NT, L], F16, tag="hT")  # reuse hT slot

        def emit_produce(bi, hd):
            """expansion -> staging -> DRAM bounce -> windows."""
            bh = HEADS * bi + hd
            stag = stagp.tile([128, 2, 4, WIN], F8, tag="stag")
            for s in range(2):
                G = gq_s if s == 0 else gk_s
                for t in range(4):
                    pe_ = psE.tile([128, 1024], F32, tag="exp")
                    nc.tensor.matmul(pe_[:, :512],
                                     facsAll[:, bh, s, 128 * t:128 * t + 128],
                                     G[:, t, :512])
                    nc.tensor.matmul(pe_[:, 512:WIN],
                                     facsAll[:, bh, s, 128 * t:128 * t + 128],
                                     G[:, t, 512:])
                    if (s * 4 + t) % 2 == 0:
                        nc.scalar.copy(out=stag[:, s, t, :], in_=pe_[:, :WIN])
                    else:
                        nc.vector.tensor_copy(out=stag[:, s, t, :], in_=pe_[:, :WIN])
            nc.gpsimd.dma_start(out=scr_d[bh], in_=stag)
            win = winp.tile([128, 2, 4, 512], F8, tag="win")
            diag = AP(scr_t, bh * SCR_BH + 127,
                      [[2 * 4 * WIN - 1, 128], [4 * WIN, 2], [WIN, 4], [1, 512]])
            nc.gpsimd.dma_start(out=win, in_=diag)
            return win

        def emit_assembly(bi, hd, win):
            pf = slice(64 * (hd % 2), 64 * (hd % 2) + 64)
            mgh = hd // 2
            toks = slice(512 * bi, 512 * bi + 512)
            qTh = qT[pf, mgh, toks]
            kTh = kT[pf, mgh, toks]
            pctx = psC.tile([65, 512], F32, tag="ctx")
            for jt in range(4):
                js = slice(128 * jt, 128 * jt + 128)
                pst = psA.tile([128, 512], F32, tag="b1")
                nc.tensor.matmul(pst, kTh[:, js], qTh, start=True, stop=False)
                nc.tensor.matmul(pst, ident8, win[:, 1, jt, :],
                                 start=False, stop=False)
                for it in range(4):
                    nc.tensor.matmul(pst[:, 128 * it:128 * it + 128],
                                     win[:, 0, it, js], ident8,
                                     start=False, stop=(it == 3))
                P = attp.tile([128, 512], F16, tag="P")
                nc.scalar.activation(out=P, in_=pst, func=AF.Exp,
                                     bias=mb_s[:, 4 * bi + jt:4 * bi + jt + 1])
                nc.tensor.matmul(pctx, vtm[:, 4 * bi + jt, hd, :], P,
                                 start=(jt == 0), stop=(jt == 3))
            rsum = attp.tile([1, 512], F16, tag="rsum")
            with nc.allow_low_precision(reason="1/softmax-sum in f16 is ample"):
                nc.vector.reciprocal(out=rsum, in_=pctx[64:65, :])
            rb = psA.tile([64, 512], F32, tag="b1")
            nc.tensor.matmul(rb, one64, rsum)
            rb_s = attp.tile([64, 512], F16, tag="rbs")
            nc.vector.tensor_copy(out=rb_s, in_=rb)
            nc.vector.tensor_mul(ctxT[pf, bi, mgh, :], pctx[0:64, :], rb_s)

        out3 = out_d.rearrange("(n p) d -> n p d", p=128)

        def emit_p3(mt):
            bi, mtb = mt // 4, mt % 4
            y = xio.tile([128, HIDDEN], F32, tag="xy")
            for nn_ in range(2):
                ns = slice(512 * nn_, 512 * nn_ + 512)
                py = psA.tile([128, 512], F32, tag="b1")
                for k in range(8):
                    nc.tensor.matmul(
                        py, ctxT[:, bi, k, 128 * mtb:128 * mtb + 128],
                        wo_s[:, k, ns], start=(k == 0), stop=(k == 7))
                nc.scalar.copy(out=y[:, ns], in_=py)
            rstd, nmr = layernorm_stats(y)
            yo = xio.tile([128, HIDDEN], F32, tag="hyo")
            nc.vector.tensor_scalar(out=yo, in0=y, scalar1=rstd, scalar2=nmr,
                                    op0=mybir.AluOpType.mult,
                                    op1=mybir.AluOpType.add)
            if with_affine:
                nc.vector.tensor_mul(yo, yo, g_s)
                nc.vector.tensor_add(yo, yo, b_s)
            nc.sync.dma_start(out=out3[mt], in_=yo)

        order = [(bi, hd) for bi in range(BLOC) for hd in range(HEADS)]
        pending = []
        for bi, hd in order:
            win = emit_produce(bi, hd)
            pending.append((bi, hd, win))
            if len(pending) > 2:
                emit_assembly(*pending.pop(0))
        for args in pending:
            emit_assembly(*args)

        for mt in range(NT):
            emit_p3(mt)

    nc.compile()
    return nc


_CACHE = {}


def _get_nc(with_bias, with_affine):
    key = (with_bias, with_affine)
    if key not in _CACHE:
        _CACHE[key] = _build(with_bias, with_affine)
    return _CACHE[key]


def _host_prep(inputs):
    hs = np.ascontiguousarray(np.asarray(inputs["hidden_states"], np.float32))
    mask = np.asarray(inputs["attention_mask"])
    rel = np.asarray(inputs["relative_embedding"], np.float32)
    wqk = np.asarray(inputs["wqk"], np.float32)
    bqk = np.asarray(inputs["bqk"], np.float32)
    wv = np.asarray(inputs["wv"], np.float32)
    bv = np.asarray(inputs["bv"], np.float32)
    wo = np.asarray(inputs["wo"], np.float32)
    bo = np.asarray(inputs["bo"], np.float32)
    ln_g = np.asarray(inputs["ln_g"], np.float32)
    ln_b = np.asarray(inputs["ln_b"], np.float32)

    assert np.all(bo == 0.0), "kernel relies on bo == 0 (softmax-in-LN cancellation)"

    with_bias = bool(np.any(bqk != 0) or np.any(bv != 0))
    with_affine = bool(np.any(ln_g != 1) or np.any(ln_b != 0))

    wqkT = np.ascontiguousarray(wqk.T).astype(np.float64)
    wqkT[:, :HIDDEN] *= SCALE
    wqkT = wqkT.astype(np.float16)
    # [mg, p, k, j] = wqkT[128k + p, 128mg + j]: contiguous per-mg weight tiles
    wqkT = np.ascontiguousarray(
        wqkT.reshape(8, 128, 16, 128).transpose(2, 1, 0, 3))
    bqk2 = bqk.astype(np.float64)
    bqk2[:HIDDEN] *= SCALE
    bqk2 = bqk2.astype(np.float16)
    wvT = np.ascontiguousarray(wv.T).astype(np.float16)
    woT = np.ascontiguousarray(wo.T).astype(np.float16)
    relT = np.zeros((HIDDEN, 64), np.float16)
    relT[:, :REL] = rel.T
    Gq, Gk = _make_G()

    shared = {"wqkT": wqkT, "wvT": wvT, "woT": woT, "relT": relT,
              "Gq": Gq, "Gk": Gk}
    if with_bias:
        shared["bqk2"] = bqk2.reshape(1, -1)
        shared["bv2"] = bv.astype(np.float16).reshape(1, -1)
        shared["ones_row"] = np.ones((1, TOK2), np.float16)
    if with_affine:
        shared["g_bcast"] = np.ascontiguousarray(
            np.broadcast_to(ln_g, (128, HIDDEN)))
        shared["b_bcast"] = np.ascontiguousarray(
            np.broadcast_to(ln_b, (128, HIDDEN)))

    in_maps = []
    for c in range(NCORES):
        m = dict(shared)
        hs_c = hs[:, 2 * c:2 * c + 2, :]
        m["hs_tok"] = np.ascontiguousarray(
            hs_c.transpose(1, 0, 2).reshape(NTOK, HIDDEN))
        mb = np.zeros((128, BLOC * 4), np.float32)
        for bi in range(BLOC):
            mrow = np.asarray(mask[2 * c + bi, 0, 0, :])
            for t in range(4):
                mb[:, 4 * bi + t] = np.where(mrow[128 * t:128 * t + 128], -1e9, 0.0)
        m["maskbias"] = mb
        in_maps.append(m)
    return in_maps, with_bias, with_affine


def kernel(**inputs):
    in_maps, with_bias, with_affine = _host_prep(inputs)
    nc = _get_nc(with_bias, with_affine)
    res = bass_utils.run_bass_kernel_spmd(nc, in_maps, core_ids=list(range(NCORES)))
    out = np.zeros((L, B, HIDDEN), np.float32)
    for c in range(NCORES):
        y = res.results[c]["out_y"]  # (NTOK, HIDDEN) token-major
        for bi in range(BLOC):
            out[:, 2 * c + bi, :] = y[512 * bi:512 * bi + 512, :]
    return out


# revision 19
# speedup vs baseline: 1.2129x; 1.2129x over previous
"""Trainium2 Bass kernel for nn_Bert_10187662426159 (DeBERTa-style
disentangled-attention BERT layer, L=512 B=16 D=1024 H=16).

Sharding: data-parallel over B - core c handles batch entries {2c, 2c+1}.

Per-core pipeline (ST orientation: scores stored [key j on partitions,
query i on free dim]; matmul operands f16, PSUM accumulation f32):
  P1  LN1 (no affine) -> h ; PE-transpose -> hT [feat, tok].  The 63
      relative-embedding rows ride as 63 extra "tokens" (cols 1024:1087
      of hT), so the q/k projection also produces qpos/kpos for free.
  P1b q/k proj (feat-major, q half pre-scaled by 1/sqrt(3*64)) over the
      1088-token axis; v proj (token-major, with a ones-column per head
      for softmax row sums).
  P2  per (batch-entry, head), software-pipelined by one head:
        qp/pk rank-63 positional factors from qT/kT slices
        window expansion [128, 640] per 128-row tile via 0/1 G-matrices
        (raw scores in delta space, f32 PSUM -> f16 staging)
        skew via DRAM bounce: write staging [128, 2, 4, 640] contiguous,
        read back with flat row stride 639 elements (one diagonal-stride
        DMA realigns every row's shift 127-p exactly - same output as a
        per-partition shift)
      assembly per 128-row j-tile in one PSUM bank: c2c matmul + pk via
      identity-matmul + qp via four identity-rhs transpose-matmuls; ONE
      ACT exp with the attention mask as a per-partition bias (-1e9) ->
      P (f16; scores bounded, exp(-1e9) = 0, no max-subtraction)
      ctx: [v | 1]^T @ P accumulates context AND row sums in PSUM;
      1/sum broadcast via a rank-1 PE matmul; DVE multiply normalizes.
  P3  y = ctxT^T @ woT, LN2 + affine.

The DeBERTa take_along_axis gathers are exact: bucket expansion is a 0/1
matmul in delta-space and the diagonal re-alignment (skew) is an exact
strided DMA read from DRAM scratch (row stride 639 on a 640-wide array
walks one element left per partition row).
"""
import contextlib
import math
import sys

import numpy as np

sys.path.insert(0, "/opt/trn_rl_repo")
sys.path.insert(0, "/opt/trn_rl_repo/concourse")

import concourse.mybir as mybir  # noqa: E402
import concourse.tile as tile  # noqa: E402
from concourse import bacc, bass_utils  # noqa: E402
from concourse.ap import AP  # noqa: E402
from concourse.masks import make_identity  # noqa: E402

F32 = mybir.dt.float32
F16 = mybir.dt.float16
F8 = mybir.dt.float8e3
F8E4 = mybir.dt.float8e4

HIDDEN, HEADS, HEAD = 1024, 16, 64
BUCKET, MAXPOS, REL = 32, 512, 63
L, B = 512, 16
EPS = 1e-7
SCALE = 1.0 / math.sqrt(3 * HEAD)
WIN = 640
NCORES = 8
BLOC = B // NCORES          # 2 batch entries per core
NTOK = L * BLOC             # 1024 tokens per core
NT = NTOK // 128            # 8 token tiles
TOK2 = NTOK + 64            # tokens + 63 rel rows + 1 pad
AF = mybir.ActivationFunctionType


def _bucket_fn(delta):
    r = np.asarray(delta)
    mid = BUCKET // 2
    abs_pos = np.where((r < mid) & (r > -mid), mid - 1,
                       np.minimum(np.abs(r), MAXPOS - 1))
    with np.errstate(divide="ignore"):
        log_pos = (np.ceil(np.log(abs_pos.astype(np.float64) / mid)
                           / math.log((MAXPOS - 1) / mid) * (mid - 1))
                   .astype(np.int64) + mid)
    bucket_pos = np.where(abs_pos <= mid, r, log_pos * np.sign(r))
    return (BUCKET - 1 + bucket_pos).astype(np.int64)


def _make_G():
    Gq, Gk = [], []
    for t in range(4):
        w0 = -127 - 128 * t
        c = np.arange(WIN)
        dq = np.clip(-(w0 + c), -511, 511)
        dk = np.clip(+(w0 + c), -511, 511)
        Gq.append(_bucket_fn(dq)[None, :] == np.arange(REL)[:, None])
        Gk.append(_bucket_fn(dk)[None, :] == np.arange(REL)[:, None])
    f8 = mybir.dt.np(F8)
    return (np.stack(Gq).transpose(1, 0, 2).astype(f8),
            np.stack(Gk).transpose(1, 0, 2).astype(f8))  # [63, 4, 640]


def _build(with_bias: bool, with_affine: bool):
    nc = bacc.Bacc("TRN2", debug=False, num_devices=NCORES)

    hs_d = nc.dram_tensor("hs_tok", (NTOK, HIDDEN), F32, kind="ExternalInput").ap()
    mb_d = nc.dram_tensor("maskbias", (128, BLOC * 4), F32, kind="ExternalInput").ap()
    wqkT_d = nc.dram_tensor("wqkT", (16, 128, 8, 128), F16, kind="ExternalInput").ap()
    wvT_d = nc.dram_tensor("wvT", (HIDDEN, HIDDEN), F16, kind="ExternalInput").ap()
    woT_d = nc.dram_tensor("woT", (HIDDEN, HIDDEN), F16, kind="ExternalInput").ap()
    relT_d = nc.dram_tensor("relT", (HIDDEN, 64), F16, kind="ExternalInput").ap()
    gq_d = nc.dram_tensor("Gq", (REL, 4, WIN), F8, kind="ExternalInput").ap()
    gk_d = nc.dram_tensor("Gk", (REL, 4, WIN), F8, kind="ExternalInput").ap()
    if with_bias:
        bqk_d = nc.dram_tensor("bqk2", (1, 2 * HIDDEN), F16, kind="ExternalInput").ap()
        bv_d = nc.dram_tensor("bv2", (1, HIDDEN), F16, kind="ExternalInput").ap()
        ones_d = nc.dram_tensor("ones_row", (1, TOK2), F16, kind="ExternalInput").ap()
    if with_affine:
        g_d = nc.dram_tensor("g_bcast", (128, HIDDEN), F32, kind="ExternalInput").ap()
        b_d = nc.dram_tensor("b_bcast", (128, HIDDEN), F32, kind="ExternalInput").ap()
    out_d = nc.dram_tensor("out_y", (NTOK, HIDDEN), F32, kind="ExternalOutput").ap()
    # DRAM scratch for the skew bounce: one [128, 2, 4, 640] f16 block per
    # (batch-entry, head).
    scr_t = nc.dram_tensor("skew_scratch", (B // NCORES * HEADS, 128, 2, 4, WIN),
                           F8, kind="Internal")
    scr_d = scr_t.ap()
    SCR_BH = 128 * 2 * 4 * WIN  # elements per (bi, head) block

    with tile.TileContext(nc) as tc, contextlib.ExitStack() as ctx:
        consts = ctx.enter_context(tc.tile_pool(name="consts", bufs=1))
        wpool = ctx.enter_context(tc.tile_pool(name="wpool", bufs=3))
        xio = ctx.enter_context(tc.tile_pool(name="xio", bufs=2))
        stat = ctx.enter_context(tc.tile_pool(name="stat", bufs=4))
        big = ctx.enter_context(tc.tile_pool(name="big", bufs=1))
        stagp = ctx.enter_context(tc.tile_pool(name="stagp", bufs=2))
        winp = ctx.enter_context(tc.tile_pool(name="winp", bufs=3))
        attp = ctx.enter_context(tc.tile_pool(name="attp", bufs=2))
        # PSUM: psA 3x1 bank (scores/fac/rb), psE 2x2 (expansion),
        # psC 1x1 (ctx) -> 8 banks total.
        psA = ctx.enter_context(tc.tile_pool(name="psA", bufs=3, space="PSUM"))
        psE = ctx.enter_context(tc.tile_pool(name="psE", bufs=2, space="PSUM"))
        psC = ctx.enter_context(tc.tile_pool(name="psC", bufs=1, space="PSUM"))

        # ---------- constants ----------
        ident16 = consts.tile([128, 128], F16)
        make_identity(nc, ident16)
        ident8 = consts.tile([128, 128], F8)
        make_identity(nc, ident8)
        eps_t = consts.tile([128, 1], F32)
        nc.vector.memset(eps_t, EPS)
        one64 = consts.tile([1, 64], F16)
        nc.vector.memset(one64, 1.0)
        gq_s = consts.tile([REL, 4, WIN], F8)
        gk_s = consts.tile([REL, 4, WIN], F8)
        mb_s = consts.tile([128, BLOC * 4], F32)
        if with_bias:
            bqk_s = consts.tile([1, 2 * HIDDEN], F16)
            bv_s = consts.tile([1, HIDDEN], F16)
            ones_s = consts.tile([1, TOK2], F16)
            nc.sync.dma_start(out=bqk_s, in_=bqk_d)
            nc.sync.dma_start(out=bv_s, in_=bv_d)
            nc.sync.dma_start(out=ones_s, in_=ones_d)
        if with_affine:
            g_s = consts.tile([128, HIDDEN], F32)
            b_s = consts.tile([128, HIDDEN], F32)
            nc.sync.dma_start(out=g_s, in_=g_d)
            nc.sync.dma_start(out=b_s, in_=b_d)

        def layernorm_stats(y):
            """-> (rstd, -mean*rstd) [128,1] tiles for the normalize apply."""
            st = stat.tile([128, 2, nc.vector.BN_STATS_DIM], F32, tag="st")
            mv = stat.tile([128, nc.vector.BN_AGGR_DIM], F32, tag="mv")
            yr = y.rearrange("p (s d) -> p s d", s=2)
            for s in range(2):
                nc.vector.bn_stats(out=st[:, s, :], in_=yr[:, s, :])
            nc.vector.bn_aggr(out=mv, in_=st)
            rstd = stat.tile([128, 1], F32, tag="rstd")
            nc.scalar.activation(out=rstd, in_=mv[:, 1:2], func=AF.Sqrt,
                                 bias=eps_t, scale=1.0)
            nc.vector.reciprocal(out=rstd, in_=rstd)
            nmr = stat.tile([128, 1], F32, tag="nmr")
            nc.vector.tensor_mul(nmr, mv[:, 0:1], rstd)
            nc.vector.tensor_scalar_mul(nmr, nmr, -1.0)
            return rstd, nmr

        # ---------- P1: LN1 + transpose (rel rows ride as extra tokens) ----
        hT = big.tile([128, NT, TOK2], F16, tag="hT")  # [feat, tok]
        hs3 = hs_d.rearrange("(n p) d -> n p d", p=128)

        def emit_ln_tile(tt):
            x = xio.tile([128, HIDDEN], F32, tag="xy")
            nc.sync.dma_start(out=x, in_=hs3[tt])
            rstd, nmr = layernorm_stats(x)
            h = xio.tile([128, HIDDEN], F16, tag="hyo")
            nc.vector.tensor_scalar(out=h, in0=x, scalar1=rstd, scalar2=nmr,
                                    op0=mybir.AluOpType.mult,
                                    op1=mybir.AluOpType.add)
            for fb in range(NT):
                ptr = psA.tile([128, 128], F16, tag="b1")
                nc.tensor.matmul(ptr, h[:, 128 * fb:128 * fb + 128], ident16,
                                 is_transpose=True)
                nc.vector.tensor_copy(out=hT[:, fb, 128 * tt:128 * tt + 128], in_=ptr)

        for tt in range(4):
            emit_ln_tile(tt)
        nc.scalar.dma_start(out=hT[:, :, NTOK:NTOK + 64],
                            in_=relT_d.rearrange("(n p) r -> p n r", p=128))
        nc.vector.memset(hT[:, :, TOK2 - 1:TOK2], 0.0)
        nc.scalar.dma_start(out=gq_s, in_=gq_d)
        nc.scalar.dma_start(out=gk_s, in_=gk_d)
        nc.scalar.dma_start(out=mb_s, in_=mb_d)

        # ---------- P1b: projections ----------
        qT = big.tile([128, NT, TOK2], F16, tag="qT")
        kT = big.tile([128, NT, TOK2], F16, tag="kT")
        vtm = big.tile([128, NT, HEADS, HEAD + 1], F16, tag="v")
        nc.vector.memset(vtm[:, :, :, HEAD:HEAD + 1], 1.0)
        wqk3 = wqkT_d

        # q/k (+qpos/kpos): feat-major out [2048 -> 16 M-tiles, 1088 tok]
        facsAll = big.tile([REL, BLOC * HEADS, 2, 512], F8, tag="facs")

        def emit_factors(hd):
            pf = slice(64 * (hd % 2), 64 * (hd % 2) + 64)
            mgh = hd // 2
            for bi in range(BLOC):
                toks = slice(512 * bi, 512 * bi + 512)
                bh = HEADS * bi + hd
                fq_ps = psA.tile([REL, 512], F32, tag="b1")
                nc.tensor.matmul(fq_ps, kT[pf, mgh, NTOK:NTOK + REL],
                                 qT[pf, mgh, toks])
                fk_ps = psA.tile([REL, 512], F32, tag="b1")
                nc.tensor.matmul(fk_ps, qT[pf, mgh, NTOK:NTOK + REL],
                                 kT[pf, mgh, toks])
                if bh % 2 == 0:
                    nc.scalar.copy(out=facsAll[:, bh, 0, :], in_=fq_ps)
                    nc.vector.tensor_copy(out=facsAll[:, bh, 1, :], in_=fk_ps)
                else:
                    nc.vector.tensor_copy(out=facsAll[:, bh, 0, :], in_=fq_ps)
                    nc.scalar.copy(out=facsAll[:, bh, 1, :], in_=fk_ps)

        NCH = [(0, 512), (512, 512), (1024, 64)]

        def emit_qk_pass(chunks, second):
            for mg in range(16):
                w_m = wpool.tile([128, 8, 128], F16, tag="wqk")
                nc.scalar.dma_start(out=w_m, in_=wqk3[mg])
                for ci in chunks:
                    n0, nw = NCH[ci]
                    ns = slice(n0, n0 + nw)
                    pq = psA.tile([128, 512], F32, tag="b1")
                    for k in range(8):
                        nc.tensor.matmul(pq[:, :nw], w_m[:, k, :], hT[:, k, ns],
                                         start=(k == 0),
                                         stop=(k == 7 and not with_bias))
                    if with_bias:
                        nc.tensor.matmul(pq[:, :nw],
                                         bqk_s[:, 128 * mg:128 * mg + 128],
                                         ones_s[:, ns], start=False, stop=True)
                    dst = qT if mg < 8 else kT
                    if (mg * 3 + ci) % 2 == 0:
                        nc.vector.tensor_copy(out=dst[:, mg % 8, ns],
                                              in_=pq[:, :nw])
                    else:
                        nc.scalar.copy(out=dst[:, mg % 8, ns], in_=pq[:, :nw])
                if second and mg >= 8:
                    emit_factors(2 * (mg - 8))
                    emit_factors(2 * (mg - 8) + 1)

        emit_qk_pass([0], False)        # tokens 0:512 (needs LN tiles 0-3)
        for tt in range(4, NT):
            emit_ln_tile(tt)
        emit_qk_pass([1, 2], True)      # tokens 512:1024 + rel, factors

        # v: token-major out [tok, feat]; wvT resident then woT reuses slot
        wv_s = big.tile([128, 8, HIDDEN], F16, tag="wvo")
        nc.sync.dma_start(out=wv_s, in_=wvT_d.rearrange("(n p) m -> p n m", p=128))
        for mt in range(NT):
            for nn_ in range(2):
                ns = slice(512 * nn_, 512 * nn_ + 512)
                pv = psA.tile([128, 512], F32, tag="b1")
                for k in range(8):
                    nc.tensor.matmul(pv, hT[:, k, 128 * mt:128 * mt + 128],
                                     wv_s[:, k, ns], start=(k == 0),
                                     stop=(k == 7 and not with_bias))
                if with_bias:
                    nc.tensor.matmul(pv, ones_s[:, 128 * mt:128 * mt + 128],
                                     bv_s[:, ns], start=False, stop=True)
                nc.vector.tensor_copy(
                    out=vtm[:, mt, 8 * nn_:8 * nn_ + 8, 0:HEAD],
                    in_=pv.rearrange("p (h d) -> p h d", d=HEAD))

        # wo load early (sync queue is idle; overlaps attention)
        wo_s = big.tile([128, 8, HIDDEN], F16, tag="wvo")  # reuse wv slot
        nc.sync.dma_start(out=wo_s, in_=woT_d.rearrange("(n p) m -> p n m", p=128))

        # ---------- P2: attention, software-pipelined by two heads ----------
        ctxT = big.tile([128, BLOC, NT, L], F16, tag="hT")  # reuse hT slot

        def emit_produce(bi, hd):
            """expansion -> staging -> DRAM bounce -> windows."""
            bh = HEADS * bi + hd
            stag = stagp.tile([128, 2, 4, WIN], F8, tag="stag")
            for s in range(2):
                G = gq_s if s == 0 else gk_s
                for t in range(4):
                    pe_ = psE.tile([128, 1024], F32, tag="exp")
                    nc.tensor.matmul(pe_[:, :512],
                                     facsAll[:, bh, s, 128 * t:128 * t + 128],
                                     G[:, t, :512])
                    nc.tensor.matmul(pe_[:, 512:WIN],
                                     facsAll[:, bh, s, 128 * t:128 * t + 128],
                                     G[:, t, 512:])
                    if (s * 4 + t) % 2 == 0:
                        nc.scalar.copy(out=stag[:, s, t, :], in_=pe_[:, :WIN])
                    else:
                        nc.vector.tensor_copy(out=stag[:, s, t, :], in_=pe_[:, :WIN])
            nc.gpsimd.dma_start(out=scr_d[bh], in_=stag)
            win = winp.tile([128, 2, 4, 512], F8, tag="win")
            diag = AP(scr_t, bh * SCR_BH + 127,
                      [[2 * 4 * WIN - 1, 128], [4 * WIN, 2], [WIN, 4], [1, 512]])
            nc.gpsimd.dma_start(out=win, in_=diag)
            return win

        def emit_assembly(bi, hd, win):
            pf = slice(64 * (hd % 2), 64 * (hd % 2) + 64)
            mgh = hd // 2
            toks = slice(512 * bi, 512 * bi + 512)
            qTh = qT[pf, mgh, toks]
            kTh = kT[pf, mgh, toks]
            pctx = psC.tile([65, 512], F32, tag="ctx")
            for jt in range(4):
                js = slice(128 * jt, 128 * jt + 128)
                pst = psA.tile([128, 512], F32, tag="b1")
                nc.tensor.matmul(pst, kTh[:, js], qTh, start=True, stop=False)
                nc.tensor.matmul(pst, ident8, win[:, 1, jt, :],
                                 start=False, stop=False)
                for it in range(4):
                    nc.tensor.matmul(pst[:, 128 * it:128 * it + 128],
                                     win[:, 0, it, js], ident8,
                                     start=False, stop=(it == 3))
                P = attp.tile([128, 512], F16, tag="P")
                nc.scalar.activation(out=P, in_=pst, func=AF.Exp,
                                     bias=mb_s[:, 4 * bi + jt:4 * bi + jt + 1])
                nc.tensor.matmul(pctx, vtm[:, 4 * bi + jt, hd, :], P,
                                 start=(jt == 0), stop=(jt == 3))
            rsum = attp.tile([1, 512], F16, tag="rsum")
            with nc.allow_low_precision(reason="1/softmax-sum in f16 is ample"):
                nc.vector.reciprocal(out=rsum, in_=pctx[64:65, :])
            rb = psA.tile([64, 512], F32, tag="b1")
            nc.tensor.matmul(rb, one64, rsum)
            rb_s = attp.tile([64, 512], F16, tag="rbs")
            nc.vector.tensor_copy(out=rb_s, in_=rb)
            nc.vector.tensor_mul(ctxT[pf, bi, mgh, :], pctx[0:64, :], rb_s)

        out3 = out_d.rearrange("(n p) d -> n p d", p=128)

        def emit_p3(mt):
            bi, mtb = mt // 4, mt % 4
            y = xio.tile([128, HIDDEN], F32, tag="xy")
            for nn_ in range(2):
                ns = slice(512 * nn_, 512 * nn_ + 512)
                py = psA.tile([128, 512], F32, tag="b1")
                for k in range(8):
                    nc.tensor.matmul(
                        py, ctxT[:, bi, k, 128 * mtb:128 * mtb + 128],
                        wo_s[:, k, ns], start=(k == 0), stop=(k == 7))
                nc.scalar.copy(out=y[:, ns], in_=py)
            rstd, nmr = layernorm_stats(y)
            yo = xio.tile([128, HIDDEN], F32, tag="hyo")
            nc.vector.tensor_scalar(out=yo, in0=y, scalar1=rstd, scalar2=nmr,
                                    op0=mybir.AluOpType.mult,
                                    op1=mybir.AluOpType.add)
            if with_affine:
                nc.vector.tensor_mul(yo, yo, g_s)
                nc.vector.tensor_add(yo, yo, b_s)
            nc.sync.dma_start(out=out3[mt], in_=yo)

        order = [(bi, hd) for bi in range(BLOC) for hd in range(HEADS)]
        pending = []
        for bi, hd in order:
            win = emit_produce(bi, hd)
            pending.append((bi, hd, win))
            if len(pending) > 2:
                emit_assembly(*pending.pop(0))
        for args in pending:
            emit_assembly(*args)

        for mt in range(NT):
            emit_p3(mt)

    nc.compile()
    return nc


_CACHE = {}


def _get_nc(with_bias, with_affine):
    key = (with_bias, with_affine)
    if key not in _CACHE:
        _CACHE[key] = _build(with_bias, with_affine)
    return _CACHE[key]


def _host_prep(inputs):
    hs = np.ascontiguousarray(np.asarray(inputs["hidden_states"], np.float32))
    mask = np.asarray(inputs["attention_mask"])
    rel = np.asarray(inputs["relative_embedding"], np.float32)
    wqk = np.asarray(inputs["wqk"], np.float32)
    bqk = np.asarray(inputs["bqk"], np.float32)
    wv = np.asarray(inputs["wv"], np.float32)
    bv = np.asarray(inputs["bv"], np.float32)
    wo = np.asarray(inputs["wo"], np.float32)
    bo = np.asarray(inputs["bo"], np.float32)
    ln_g = np.asarray(inputs["ln_g"], np.float32)
    ln_b = np.asarray(inputs["ln_b"], np.float32)

    assert np.all(bo == 0.0), "kernel relies on bo == 0 (softmax-in-LN cancellation)"

    with_bias = bool(np.any(bqk != 0) or np.any(bv != 0))
    with_affine = bool(np.any(ln_g != 1) or np.any(ln_b != 0))

    wqkT = np.ascontiguousarray(wqk.T).astype(np.float64)
    wqkT[:, :HIDDEN] *= SCALE
    wqkT = wqkT.astype(np.float16)
    # [mg, p, k, j] = wqkT[128k + p, 128mg + j]: contiguous per-mg weight tiles
    wqkT = np.ascontiguousarray(
        wqkT.reshape(8, 128, 16, 128).transpose(2, 1, 0, 3))
    bqk2 = bqk.astype(np.float64)
    bqk2[:HIDDEN] *= SCALE
    bqk2 = bqk2.astype(np.float16)
    wvT = np.ascontiguousarray(wv.T).astype(np.float16)
    woT = np.ascontiguousarray(wo.T).astype(np.float16)
    relT = np.zeros((HIDDEN, 64), np.float16)
    relT[:, :REL] = rel.T
    Gq, Gk = _make_G()

    shared = {"wqkT": wqkT, "wvT": wvT, "woT": woT, "relT": relT,
              "Gq": Gq, "Gk": Gk}
    if with_bias:
        shared["bqk2"] = bqk2.reshape(1, -1)
        shared["bv2"] = bv.astype(np.float16).reshape(1, -1)
        shared["ones_row"] = np.ones((1, TOK2), np.float16)
    if with_affine:
        shared["g_bcast"] = np.ascontiguousarray(
            np.broadcast_to(ln_g, (128, HIDDEN)))
        shared["b_bcast"] = np.ascontiguousarray(
            np.broadcast_to(ln_b, (128, HIDDEN)))

    in_maps = []
    for c in range(NCORES):
        m = dict(shared)
        hs_c = hs[:, 2 * c:2 * c + 2, :]
        m["hs_tok"] = np.ascontiguousarray(
            hs_c.transpose(1, 0, 2).reshape(NTOK, HIDDEN))
        mb = np.zeros((128, BLOC * 4), np.float32)
        for bi in range(BLOC):
            mrow = np.asarray(mask[2 * c + bi, 0, 0, :])
            for t in range(4):
                mb[:, 4 * bi + t] = np.where(mrow[128 * t:128 * t + 128], -1e9, 0.0)
        m["maskbias"] = mb
        in_maps.append(m)
    return in_maps, with_bias, with_affine


def kernel(**inputs):
    in_maps, with_bias, with_affine = _host_prep(inputs)
    nc = _get_nc(with_bias, with_affine)
    res = bass_utils.run_bass_kernel_spmd(nc, in_maps, core_ids=list(range(NCORES)))
    out = np.zeros((L, B, HIDDEN), np.float32)
    for c in range(NCORES):
        y = res.results[c]["out_y"]  # (NTOK, HIDDEN) token-major
        for bi in range(BLOC):
            out[:, 2 * c + bi, :] = y[512 * bi:512 * bi + 512, :]
    return out


# revision 22
# speedup vs baseline: 1.4943x; 1.2321x over previous
"""Trainium2 Bass kernel for nn_Bert_10187662426159 (DeBERTa-style
disentangled-attention BERT layer, L=512 B=16 D=1024 H=16).

Sharding: data-parallel over B - core c handles batch entries {2c, 2c+1}.

Per-core pipeline (ST orientation: scores stored [key j on partitions,
query i on free dim]; matmul operands f16, PSUM accumulation f32):
  P1  LN1 (no affine) -> h ; PE-transpose -> hT [feat, tok].  The 63
      relative-embedding rows ride as 63 extra "tokens" (cols 1024:1087
      of hT), so the q/k projection also produces qpos/kpos for free.
  P1b q/k proj (feat-major, q half pre-scaled by 1/sqrt(3*64)) over the
      1088-token axis; v proj (token-major, with a ones-column per head
      for softmax row sums).
  P2  per (batch-entry, head), software-pipelined by one head:
        qp/pk rank-63 positional factors from qT/kT slices
        window expansion [128, 640] per 128-row tile via 0/1 G-matrices
        (raw scores in delta space, f32 PSUM -> f16 staging)
        skew via DRAM bounce: write staging [128, 2, 4, 640] contiguous,
        read back with flat row stride 639 elements (one diagonal-stride
        DMA realigns every row's shift 127-p exactly - same output as a
        per-partition shift)
      assembly per 128-row j-tile in one PSUM bank: c2c matmul + pk via
      identity-matmul + qp via four identity-rhs transpose-matmuls; ONE
      ACT exp with the attention mask as a per-partition bias (-1e9) ->
      P (f16; scores bounded, exp(-1e9) = 0, no max-subtraction)
      ctx: [v | 1]^T @ P accumulates context AND row sums in PSUM;
      1/sum broadcast via a rank-1 PE matmul; DVE multiply normalizes.
  P3  y = ctxT^T @ woT, LN2 + affine.

The DeBERTa take_along_axis gathers are exact: bucket expansion is a 0/1
matmul in delta-space and the diagonal re-alignment (skew) is an exact
strided DMA read from DRAM scratch (row stride 639 on a 640-wide array
walks one element left per partition row).
"""
import contextlib
import math
import sys

import numpy as np

sys.path.insert(0, "/opt/trn_rl_repo")
sys.path.insert(0, "/opt/trn_rl_repo/concourse")

import concourse.mybir as mybir  # noqa: E402
import concourse.tile as tile  # noqa: E402
from concourse import bacc, bass_utils  # noqa: E402
from concourse.ap import AP  # noqa: E402
from concourse.masks import make_identity  # noqa: E402

F32 = mybir.dt.float32
F16 = mybir.dt.float16
F8 = mybir.dt.float8e3
F8E4 = mybir.dt.float8e4

HIDDEN, HEADS, HEAD = 1024, 16, 64
BUCKET, MAXPOS, REL = 32, 512, 63
L, B = 512, 16
EPS = 1e-7
SCALE = 1.0 / math.sqrt(3 * HEAD)
WIN = 640
NCORES = 8
BLOC = B // NCORES          # 2 batch entries per core
NTOK = L * BLOC             # 1024 tokens per core
NT = NTOK // 128            # 8 token tiles
TOK2 = NTOK + 64            # tokens + 63 rel rows + 1 pad
AF = mybir.ActivationFunctionType


def _bucket_fn(delta):
    r = np.asarray(delta)
    mid = BUCKET // 2
    abs_pos = np.where((r < mid) & (r > -mid), mid - 1,
                       np.minimum(np.abs(r), MAXPOS - 1))
    with np.errstate(divide="ignore"):
        log_pos = (np.ceil(np.log(abs_pos.astype(np.float64) / mid)
                           / math.log((MAXPOS - 1) / mid) * (mid - 1))
                   .astype(np.int64) + mid)
    bucket_pos = np.where(abs_pos <= mid, r, log_pos * np.sign(r))
    return (BUCKET - 1 + bucket_pos).astype(np.int64)


def _make_G():
    Gq, Gk = [], []
    for t in range(4):
        w0 = -127 - 128 * t
        c = np.arange(WIN)
        dq = np.clip(-(w0 + c), -511, 511)
        dk = np.clip(+(w0 + c), -511, 511)
        Gq.append(_bucket_fn(dq)[None, :] == np.arange(REL)[:, None])
        Gk.append(_bucket_fn(dk)[None, :] == np.arange(REL)[:, None])
    f8 = mybir.dt.np(F8)
    return (np.stack(Gq).transpose(1, 0, 2).astype(f8),
            np.stack(Gk).transpose(1, 0, 2).astype(f8))  # [63, 4, 640]


def _build(with_bias: bool, with_affine: bool):
    nc = bacc.Bacc("TRN2", debug=False, num_devices=NCORES)

    hs_d = nc.dram_tensor("hs_tok", (NTOK, HIDDEN), F32, kind="ExternalInput").ap()
    mb_d = nc.dram_tensor("maskbias", (128, BLOC * 4), F32, kind="ExternalInput").ap()
    wqkT_d = nc.dram_tensor("wqkT", (16, 128, 8, 128), F16, kind="ExternalInput").ap()
    wvT_d = nc.dram_tensor("wvT", (HIDDEN, HIDDEN), F16, kind="ExternalInput").ap()
    woT_d = nc.dram_tensor("woT", (HIDDEN, HIDDEN), F16, kind="ExternalInput").ap()
    relT_d = nc.dram_tensor("relT", (HIDDEN, 64), F16, kind="ExternalInput").ap()
    gq_d = nc.dram_tensor("Gq", (REL, 4, WIN), F8, kind="ExternalInput").ap()
    gk_d = nc.dram_tensor("Gk", (REL, 4, WIN), F8, kind="ExternalInput").ap()
    if with_bias:
        bqk_d = nc.dram_tensor("bqk2", (1, 2 * HIDDEN), F16, kind="ExternalInput").ap()
        bv_d = nc.dram_tensor("bv2", (1, HIDDEN), F16, kind="ExternalInput").ap()
        ones_d = nc.dram_tensor("ones_row", (1, TOK2), F16, kind="ExternalInput").ap()
    if with_affine:
        g_d = nc.dram_tensor("g_bcast", (128, HIDDEN), F32, kind="ExternalInput").ap()
        b_d = nc.dram_tensor("b_bcast", (128, HIDDEN), F32, kind="ExternalInput").ap()
    out_d = nc.dram_tensor("out_y", (NTOK, HIDDEN), F32, kind="ExternalOutput").ap()
    # DRAM scratch for the skew bounce: one [128, 2, 2, 4, 640] f8 block per
    # (batch-entry, head-pair).
    scr_t = nc.dram_tensor("skew_scratch",
                           (B // NCORES * HEADS // 2, 128, 2, 2, 4, WIN),
                           F8, kind="Internal")
    scr_d = scr_t.ap()
    SCR_PR = 128 * 2 * 2 * 4 * WIN  # elements per (bi, head-pair) block

    with tile.TileContext(nc) as tc, contextlib.ExitStack() as ctx:
        consts = ctx.enter_context(tc.tile_pool(name="consts", bufs=1))
        wpool = ctx.enter_context(tc.tile_pool(name="wpool", bufs=3))
        xio = ctx.enter_context(tc.tile_pool(name="xio", bufs=2))
        stat = ctx.enter_context(tc.tile_pool(name="stat", bufs=4))
        big = ctx.enter_context(tc.tile_pool(name="big", bufs=1))
        stagp = ctx.enter_context(tc.tile_pool(name="stagp", bufs=2))
        winp = ctx.enter_context(tc.tile_pool(name="winp", bufs=3))
        attp = ctx.enter_context(tc.tile_pool(name="attp", bufs=2))
        # PSUM: psA 3x1 bank (scores/fac/rb), psE 2x2 (expansion),
        # psC 1x1 (ctx) -> 8 banks total.
        psA = ctx.enter_context(tc.tile_pool(name="psA", bufs=3, space="PSUM"))
        psE = ctx.enter_context(tc.tile_pool(name="psE", bufs=2, space="PSUM"))
        psC = ctx.enter_context(tc.tile_pool(name="psC", bufs=1, space="PSUM"))

        # ---------- constants ----------
        ident16 = consts.tile([128, 128], F16)
        make_identity(nc, ident16)
        ident8 = consts.tile([128, 128], F8)
        make_identity(nc, ident8)
        eps_t = consts.tile([128, 1], F32)
        nc.vector.memset(eps_t, EPS)
        one64 = consts.tile([1, 64], F16)
        nc.vector.memset(one64, 1.0)
        gq_s = consts.tile([REL, 4, WIN], F8)
        gk_s = consts.tile([REL, 4, WIN], F8)
        mb_s = consts.tile([128, BLOC * 4], F32)
        if with_bias:
            bqk_s = consts.tile([1, 2 * HIDDEN], F16)
            bv_s = consts.tile([1, HIDDEN], F16)
            ones_s = consts.tile([1, TOK2], F16)
            nc.sync.dma_start(out=bqk_s, in_=bqk_d)
            nc.sync.dma_start(out=bv_s, in_=bv_d)
            nc.sync.dma_start(out=ones_s, in_=ones_d)
        if with_affine:
            g_s = consts.tile([128, HIDDEN], F32)
            b_s = consts.tile([128, HIDDEN], F32)
            nc.sync.dma_start(out=g_s, in_=g_d)
            nc.sync.dma_start(out=b_s, in_=b_d)

        def layernorm_stats(y):
            """-> (rstd, -mean*rstd) [128,1] tiles for the normalize apply."""
            st = stat.tile([128, 2, nc.vector.BN_STATS_DIM], F32, tag="st")
            mv = stat.tile([128, nc.vector.BN_AGGR_DIM], F32, tag="mv")
            yr = y.rearrange("p (s d) -> p s d", s=2)
            for s in range(2):
                nc.vector.bn_stats(out=st[:, s, :], in_=yr[:, s, :])
            nc.vector.bn_aggr(out=mv, in_=st)
            rstd = stat.tile([128, 1], F32, tag="rstd")
            nc.scalar.activation(out=rstd, in_=mv[:, 1:2], func=AF.Sqrt,
                                 bias=eps_t, scale=1.0)
            nc.vector.reciprocal(out=rstd, in_=rstd)
            nmr = stat.tile([128, 1], F32, tag="nmr")
            nc.vector.tensor_mul(nmr, mv[:, 0:1], rstd)
            nc.vector.tensor_scalar_mul(nmr, nmr, -1.0)
            return rstd, nmr

        # ---------- P1: LN1 + transpose (rel rows ride as extra tokens) ----
        hT = big.tile([128, NT, TOK2], F16, tag="hT")  # [feat, tok]
        hs3 = hs_d.rearrange("(n p) d -> n p d", p=128)

        def emit_ln_tile(tt):
            x = xio.tile([128, HIDDEN], F32, tag="xy")
            nc.sync.dma_start(out=x, in_=hs3[tt])
            rstd, nmr = layernorm_stats(x)
            h = xio.tile([128, HIDDEN], F16, tag="hyo")
            nc.vector.tensor_scalar(out=h, in0=x, scalar1=rstd, scalar2=nmr,
                                    op0=mybir.AluOpType.mult,
                                    op1=mybir.AluOpType.add)
            for fb in range(NT):
                ptr = psA.tile([128, 128], F16, tag="b1")
                nc.tensor.matmul(ptr, h[:, 128 * fb:128 * fb + 128], ident16,
                                 is_transpose=True)
                if fb % 2 == 0:
                    nc.scalar.copy(out=hT[:, fb, 128 * tt:128 * tt + 128], in_=ptr)
                else:
                    nc.vector.tensor_copy(out=hT[:, fb, 128 * tt:128 * tt + 128],
                                          in_=ptr)

        for tt in range(4):
            emit_ln_tile(tt)
        nc.scalar.dma_start(out=hT[:, :, NTOK:NTOK + 64],
                            in_=relT_d.rearrange("(n p) r -> p n r", p=128))
        nc.vector.memset(hT[:, :, TOK2 - 1:TOK2], 0.0)
        nc.scalar.dma_start(out=gq_s, in_=gq_d)
        nc.scalar.dma_start(out=gk_s, in_=gk_d)
        nc.scalar.dma_start(out=mb_s, in_=mb_d)

        # ---------- P1b: projections ----------
        qT = big.tile([128, NT, TOK2], F16, tag="qT")
        kT = big.tile([128, NT, TOK2], F16, tag="kT")
        vtm = big.tile([128, NT, HEADS, HEAD + 1], F16, tag="v")
        nc.vector.memset(vtm[:, :, :, HEAD:HEAD + 1], 1.0)
        wqk3 = wqkT_d

        # q/k (+qpos/kpos): feat-major out [2048 -> 16 M-tiles, 1088 tok]
        facsAll = big.tile([REL, BLOC * HEADS, 2, 512], F8, tag="facs")

        def emit_factors(hd):
            pf = slice(64 * (hd % 2), 64 * (hd % 2) + 64)
            mgh = hd // 2
            for bi in range(BLOC):
                toks = slice(512 * bi, 512 * bi + 512)
                bh = HEADS * bi + hd
                fq_ps = psA.tile([REL, 512], F32, tag="b1")
                nc.tensor.matmul(fq_ps, kT[pf, mgh, NTOK:NTOK + REL],
                                 qT[pf, mgh, toks])
                fk_ps = psA.tile([REL, 512], F32, tag="b1")
                nc.tensor.matmul(fk_ps, qT[pf, mgh, NTOK:NTOK + REL],
                                 kT[pf, mgh, toks])
                if bh % 2 == 0:
                    nc.scalar.copy(out=facsAll[:, bh, 0, :], in_=fq_ps)
                    nc.vector.tensor_copy(out=facsAll[:, bh, 1, :], in_=fk_ps)
                else:
                    nc.vector.tensor_copy(out=facsAll[:, bh, 0, :], in_=fq_ps)
                    nc.scalar.copy(out=facsAll[:, bh, 1, :], in_=fk_ps)

        NCH = [(0, 512), (512, 512), (1024, 64)]

        def emit_qk_pass(chunks, second):
            for mg in range(16):
                w_m = wpool.tile([128, 8, 128], F16, tag="wqk")
                nc.scalar.dma_start(out=w_m, in_=wqk3[mg])
                for ci in chunks:
                    n0, nw = NCH[ci]
                    ns = slice(n0, n0 + nw)
                    pq = psA.tile([128, 512], F32, tag="b1")
                    for k in range(8):
                        nc.tensor.matmul(pq[:, :nw], w_m[:, k, :], hT[:, k, ns],
                                         start=(k == 0),
                                         stop=(k == 7 and not with_bias))
                    if with_bias:
                        nc.tensor.matmul(pq[:, :nw],
                                         bqk_s[:, 128 * mg:128 * mg + 128],
                                         ones_s[:, ns], start=False, stop=True)
                    dst = qT if mg < 8 else kT
                    if (mg * 3 + ci) % 2 == 0:
                        nc.vector.tensor_copy(out=dst[:, mg % 8, ns],
                                              in_=pq[:, :nw])
                    else:
                        nc.scalar.copy(out=dst[:, mg % 8, ns], in_=pq[:, :nw])
                if second and mg >= 8:
                    emit_factors(2 * (mg - 8))
                    emit_factors(2 * (mg - 8) + 1)

        emit_qk_pass([0], False)        # tokens 0:512 (needs LN tiles 0-3)
        for tt in range(4, NT):
            emit_ln_tile(tt)
        emit_qk_pass([1, 2], True)      # tokens 512:1024 + rel, factors

        # v: token-major out [tok, feat]; wvT resident then woT reuses slot
        wv_s = big.tile([128, 8, HIDDEN], F16, tag="wvo")
        nc.sync.dma_start(out=wv_s, in_=wvT_d.rearrange("(n p) m -> p n m", p=128))
        for mt in range(NT):
            for nn_ in range(2):
                ns = slice(512 * nn_, 512 * nn_ + 512)
                pv = psA.tile([128, 512], F32, tag="b1")
                for k in range(8):
                    nc.tensor.matmul(pv, hT[:, k, 128 * mt:128 * mt + 128],
                                     wv_s[:, k, ns], start=(k == 0),
                                     stop=(k == 7 and not with_bias))
                if with_bias:
                    nc.tensor.matmul(pv, ones_s[:, 128 * mt:128 * mt + 128],
                                     bv_s[:, ns], start=False, stop=True)
                nc.vector.tensor_copy(
                    out=vtm[:, mt, 8 * nn_:8 * nn_ + 8, 0:HEAD],
                    in_=pv.rearrange("p (h d) -> p h d", d=HEAD))

        # wo load early (sync queue is idle; overlaps attention)
        wo_s = big.tile([128, 8, HIDDEN], F16, tag="wvo")  # reuse wv slot
        nc.sync.dma_start(out=wo_s, in_=woT_d.rearrange("(n p) m -> p n m", p=128))

        # ---------- P2: attention, software-pipelined by two heads ----------
        ctxT = big.tile([128, BLOC, NT, L], F16, tag="hT")  # reuse hT slot

        def emit_produce(bi, hp):
            """two heads' expansions -> staging -> one DRAM bounce -> windows."""
            pr = (HEADS * bi) // 2 + hp
            stag = stagp.tile([128, 2, 2, 4, WIN], F8, tag="stag")
            for hh in range(2):
                bh = HEADS * bi + 2 * hp + hh
                for s in range(2):
                    G = gq_s if s == 0 else gk_s
                    for t in range(4):
                        pe_ = psE.tile([128, 1024], F32, tag="exp")
                        nc.tensor.matmul(pe_[:, :512],
                                         facsAll[:, bh, s, 128 * t:128 * t + 128],
                                         G[:, t, :512])
                        nc.tensor.matmul(pe_[:, 512:WIN],
                                         facsAll[:, bh, s, 128 * t:128 * t + 128],
                                         G[:, t, 512:])
                        if (s * 4 + t) % 2 == 0:
                            nc.scalar.copy(out=stag[:, hh, s, t, :],
                                           in_=pe_[:, :WIN])
                        else:
                            nc.vector.tensor_copy(out=stag[:, hh, s, t, :],
                                                  in_=pe_[:, :WIN])
            nc.gpsimd.dma_start(out=scr_d[pr], in_=stag)
            win = winp.tile([128, 2, 2, 4, 512], F8, tag="win")
            # (hh, s) flatten to one dim: strides nest exactly (5120 = 2*2560)
            diag = AP(scr_t, pr * SCR_PR + 127,
                      [[2 * 2 * 4 * WIN - 1, 128], [4 * WIN, 4], [WIN, 4], [1, 512]])
            nc.gpsimd.dma_start(out=win.rearrange("p h s t j -> p (h s) t j"),
                                in_=diag)
            return win

        def emit_assembly(bi, hd, win):
            win = win[:, hd % 2]
            pf = slice(64 * (hd % 2), 64 * (hd % 2) + 64)
            mgh = hd // 2
            toks = slice(512 * bi, 512 * bi + 512)
            qTh = qT[pf, mgh, toks]
            kTh = kT[pf, mgh, toks]
            pctx = psC.tile([65, 512], F32, tag="ctx")
            for jt in range(4):
                js = slice(128 * jt, 128 * jt + 128)
                pst = psA.tile([128, 512], F32, tag="b1")
                nc.tensor.matmul(pst, kTh[:, js], qTh, start=True, stop=False)
                nc.tensor.matmul(pst, ident8, win[:, 1, jt, :],
                                 start=False, stop=False)
                for it in range(4):
                    nc.tensor.matmul(pst[:, 128 * it:128 * it + 128],
                                     win[:, 0, it, js], ident8,
                                     start=False, stop=(it == 3))
                P = attp.tile([128, 512], F16, tag="P")
                nc.scalar.activation(out=P, in_=pst, func=AF.Exp,
                                     bias=mb_s[:, 4 * bi + jt:4 * bi + jt + 1])
                nc.tensor.matmul(pctx, vtm[:, 4 * bi + jt, hd, :], P,
                                 start=(jt == 0), stop=(jt == 3))
            rsum = attp.tile([1, 512], F16, tag="rsum")
            with nc.allow_low_precision(reason="1/softmax-sum in f16 is ample"):
                nc.vector.reciprocal(out=rsum, in_=pctx[64:65, :])
            rb = psA.tile([64, 512], F32, tag="b1")
            nc.tensor.matmul(rb, one64, rsum)
            rb_s = attp.tile([64, 512], F16, tag="rbs")
            nc.vector.tensor_copy(out=rb_s, in_=rb)
            nc.vector.tensor_mul(ctxT[pf, bi, mgh, :], pctx[0:64, :], rb_s)

        out3 = out_d.rearrange("(n p) d -> n p d", p=128)

        def emit_p3(mt):
            bi, mtb = mt // 4, mt % 4
            y = xio.tile([128, HIDDEN], F32, tag="xy")
            for nn_ in range(2):
                ns = slice(512 * nn_, 512 * nn_ + 512)
                py = psA.tile([128, 512], F32, tag="b1")
                for k in range(8):
                    nc.tensor.matmul(
                        py, ctxT[:, bi, k, 128 * mtb:128 * mtb + 128],
                        wo_s[:, k, ns], start=(k == 0), stop=(k == 7))
                nc.scalar.copy(out=y[:, ns], in_=py)
            rstd, nmr = layernorm_stats(y)
            yo = xio.tile([128, HIDDEN], F32, tag="hyo")
            nc.vector.tensor_scalar(out=yo, in0=y, scalar1=rstd, scalar2=nmr,
                                    op0=mybir.AluOpType.mult,
                                    op1=mybir.AluOpType.add)
            if with_affine:
                nc.vector.tensor_mul(yo, yo, g_s)
                nc.vector.tensor_add(yo, yo, b_s)
            nc.sync.dma_start(out=out3[mt], in_=yo)

        order = [(bi, hp) for bi in range(BLOC) for hp in range(HEADS // 2)]
        pending = []
        for bi, hp in order:
            win = emit_produce(bi, hp)
            pending.append((bi, hp, win))
            if len(pending) > 2:
                pbi, php, pwin = pending.pop(0)
                emit_assembly(pbi, 2 * php, pwin)
                emit_assembly(pbi, 2 * php + 1, pwin)
        for pbi, php, pwin in pending:
            emit_assembly(pbi, 2 * php, pwin)
            emit_assembly(pbi, 2 * php + 1, pwin)

        for mt in range(NT):
            emit_p3(mt)

    nc.compile()
    return nc


_CACHE = {}


def _get_nc(with_bias, with_affine):
    key = (with_bias, with_affine)
    if key not in _CACHE:
        _CACHE[key] = _build(with_bias, with_affine)
    return _CACHE[key]


def _host_prep(inputs):
    hs = np.ascontiguousarray(np.asarray(inputs["hidden_states"], np.float32))
    mask = np.asarray(inputs["attention_mask"])
    rel = np.asarray(inputs["relative_embedding"], np.float32)
    wqk = np.asarray(inputs["wqk"], np.float32)
    bqk = np.asarray(inputs["bqk"], np.float32)
    wv = np.asarray(inputs["wv"], np.float32)
    bv = np.asarray(inputs["bv"], np.float32)
    wo = np.asarray(inputs["wo"], np.float32)
    bo = np.asarray(inputs["bo"], np.float32)
    ln_g = np.asarray(inputs["ln_g"], np.float32)
    ln_b = np.asarray(inputs["ln_b"], np.float32)

    assert np.all(bo == 0.0), "kernel relies on bo == 0 (softmax-in-LN cancellation)"

    with_bias = bool(np.any(bqk != 0) or np.any(bv != 0))
    with_affine = bool(np.any(ln_g != 1) or np.any(ln_b != 0))

    wqkT = np.ascontiguousarray(wqk.T).astype(np.float64)
    wqkT[:, :HIDDEN] *= SCALE
    wqkT = wqkT.astype(np.float16)
    # [mg, p, k, j] = wqkT[128k + p, 128mg + j]: contiguous per-mg weight tiles
    wqkT = np.ascontiguousarray(
        wqkT.reshape(8, 128, 16, 128).transpose(2, 1, 0, 3))
    bqk2 = bqk.astype(np.float64)
    bqk2[:HIDDEN] *= SCALE
    bqk2 = bqk2.astype(np.float16)
    wvT = np.ascontiguousarray(wv.T).astype(np.float16)
    woT = np.ascontiguousarray(wo.T).astype(np.float16)
    relT = np.zeros((HIDDEN, 64), np.float16)
    relT[:, :REL] = rel.T
    Gq, Gk = _make_G()

    shared = {"wqkT": wqkT, "wvT": wvT, "woT": woT, "relT": relT,
              "Gq": Gq, "Gk": Gk}
    if with_bias:
        shared["bqk2"] = bqk2.reshape(1, -1)
        shared["bv2"] = bv.astype(np.float16).reshape(1, -1)
        shared["ones_row"] = np.ones((1, TOK2), np.float16)
    if with_affine:
        shared["g_bcast"] = np.ascontiguousarray(
            np.broadcast_to(ln_g, (128, HIDDEN)))
        shared["b_bcast"] = np.ascontiguousarray(
            np.broadcast_to(ln_b, (128, HIDDEN)))

    in_maps = []
    for c in range(NCORES):
        m = dict(shared)
        hs_c = hs[:, 2 * c:2 * c + 2, :]
        m["hs_tok"] = np.ascontiguousarray(
            hs_c.transpose(1, 0, 2).reshape(NTOK, HIDDEN))
        mb = np.zeros((128, BLOC * 4), np.float32)
        for bi in range(BLOC):
            mrow = np.asarray(mask[2 * c + bi, 0, 0, :])
            for t in range(4):
                mb[:, 4 * bi + t] = np.where(mrow[128 * t:128 * t + 128], -1e9, 0.0)
        m["maskbias"] = mb
        in_maps.append(m)
    return in_maps, with_bias, with_affine


def kernel(**inputs):
    in_maps, with_bias, with_affine = _host_prep(inputs)
    nc = _get_nc(with_bias, with_affine)
    res = bass_utils.run_bass_kernel_spmd(nc, in_maps, core_ids=list(range(NCORES)))
    out = np.zeros((L, B, HIDDEN), np.float32)
    for c in range(NCORES):
        y = res.results[c]["out_y"]  # (NTOK, HIDDEN) token-major
        for bi in range(BLOC):
            out[:, 2 * c + bi, :] = y[512 * bi:512 * bi + 512, :]
    return out


# revision 23
# speedup vs baseline: 1.6966x; 1.1353x over previous
"""Trainium2 Bass kernel for nn_Bert_10187662426159 (DeBERTa-style
disentangled-attention BERT layer, L=512 B=16 D=1024 H=16).

Sharding: data-parallel over B - core c handles batch entries {2c, 2c+1}.

Per-core pipeline (ST orientation: scores stored [key j on partitions,
query i on free dim]; matmul operands f16, PSUM accumulation f32):
  P1  LN1 (no affine) -> h ; PE-transpose -> hT [feat, tok].  The 63
      relative-embedding rows ride as 63 extra "tokens" (cols 1024:1087
      of hT), so the q/k projection also produces qpos/kpos for free.
  P1b q/k proj (feat-major, q half pre-scaled by 1/sqrt(3*64)) over the
      1088-token axis; v proj (token-major, with a ones-column per head
      for softmax row sums).
  P2  per (batch-entry, head), software-pipelined by one head:
        qp/pk rank-63 positional factors from qT/kT slices
        window expansion [128, 640] per 128-row tile via 0/1 G-matrices
        (raw scores in delta space, f32 PSUM -> f16 staging)
        skew via DRAM bounce: write staging [128, 2, 4, 640] contiguous,
        read back with flat row stride 639 elements (one diagonal-stride
        DMA realigns every row's shift 127-p exactly - same output as a
        per-partition shift)
      assembly per 128-row j-tile in one PSUM bank: c2c matmul + pk via
      identity-matmul + qp via four identity-rhs transpose-matmuls; ONE
      ACT exp with the attention mask as a per-partition bias (-1e9) ->
      P (f16; scores bounded, exp(-1e9) = 0, no max-subtraction)
      ctx: [v | 1]^T @ P accumulates context AND row sums in PSUM;
      1/sum broadcast via a rank-1 PE matmul; DVE multiply normalizes.
  P3  y = ctxT^T @ woT, LN2 + affine.

The DeBERTa take_along_axis gathers are exact: bucket expansion is a 0/1
matmul in delta-space and the diagonal re-alignment (skew) is an exact
strided DMA read from DRAM scratch (row stride 639 on a 640-wide array
walks one element left per partition row).
"""
import contextlib
import math
import sys

import numpy as np

sys.path.insert(0, "/opt/trn_rl_repo")
sys.path.insert(0, "/opt/trn_rl_repo/concourse")

import concourse.mybir as mybir  # noqa: E402
import concourse.tile as tile  # noqa: E402
from concourse import bacc, bass_utils  # noqa: E402
from concourse.ap import AP  # noqa: E402
from concourse.masks import make_identity  # noqa: E402

F32 = mybir.dt.float32
F16 = mybir.dt.float16
F8 = mybir.dt.float8e3
F8E4 = mybir.dt.float8e4

HIDDEN, HEADS, HEAD = 1024, 16, 64
BUCKET, MAXPOS, REL = 32, 512, 63
L, B = 512, 16
EPS = 1e-7
SCALE = 1.0 / math.sqrt(3 * HEAD)
WIN = 640
NCORES = 8
BLOC = B // NCORES          # 2 batch entries per core
NTOK = L * BLOC             # 1024 tokens per core
NT = NTOK // 128            # 8 token tiles
TOK2 = NTOK + 64            # tokens + 63 rel rows + 1 pad
AF = mybir.ActivationFunctionType


def _bucket_fn(delta):
    r = np.asarray(delta)
    mid = BUCKET // 2
    abs_pos = np.where((r < mid) & (r > -mid), mid - 1,
                       np.minimum(np.abs(r), MAXPOS - 1))
    with np.errstate(divide="ignore"):
        log_pos = (np.ceil(np.log(abs_pos.astype(np.float64) / mid)
                           / math.log((MAXPOS - 1) / mid) * (mid - 1))
                   .astype(np.int64) + mid)
    bucket_pos = np.where(abs_pos <= mid, r, log_pos * np.sign(r))
    return (BUCKET - 1 + bucket_pos).astype(np.int64)


def _make_G():
    Gq, Gk = [], []
    for t in range(4):
        w0 = -127 - 128 * t
        c = np.arange(WIN)
        dq = np.clip(-(w0 + c), -511, 511)
        dk = np.clip(+(w0 + c), -511, 511)
        Gq.append(_bucket_fn(dq)[None, :] == np.arange(REL)[:, None])
        Gk.append(_bucket_fn(dk)[None, :] == np.arange(REL)[:, None])
    f8 = mybir.dt.np(F8)
    return (np.stack(Gq).transpose(1, 0, 2).astype(f8),
            np.stack(Gk).transpose(1, 0, 2).astype(f8))  # [63, 4, 640]


def _build(with_bias: bool, with_affine: bool):
    nc = bacc.Bacc("TRN2", debug=False, num_devices=NCORES)

    hs_d = nc.dram_tensor("hs_tok", (NTOK, HIDDEN), F32, kind="ExternalInput").ap()
    mb_d = nc.dram_tensor("maskbias", (128, BLOC * 4), F32, kind="ExternalInput").ap()
    wqkT_d = nc.dram_tensor("wqkT", (16, 128, 8, 128), F16, kind="ExternalInput").ap()
    wvT_d = nc.dram_tensor("wvT", (HIDDEN, HIDDEN), F16, kind="ExternalInput").ap()
    woT_d = nc.dram_tensor("woT", (HIDDEN, HIDDEN), F16, kind="ExternalInput").ap()
    relT_d = nc.dram_tensor("relT", (HIDDEN, 64), F16, kind="ExternalInput").ap()
    gq_d = nc.dram_tensor("Gq", (REL, 4, WIN), F8, kind="ExternalInput").ap()
    gk_d = nc.dram_tensor("Gk", (REL, 4, WIN), F8, kind="ExternalInput").ap()
    if with_bias:
        bqk_d = nc.dram_tensor("bqk2", (1, 2 * HIDDEN), F16, kind="ExternalInput").ap()
        bv_d = nc.dram_tensor("bv2", (1, HIDDEN), F16, kind="ExternalInput").ap()
        ones_d = nc.dram_tensor("ones_row", (1, TOK2), F16, kind="ExternalInput").ap()
    if with_affine:
        g_d = nc.dram_tensor("g_bcast", (128, HIDDEN), F32, kind="ExternalInput").ap()
        b_d = nc.dram_tensor("b_bcast", (128, HIDDEN), F32, kind="ExternalInput").ap()
    out_d = nc.dram_tensor("out_y", (NTOK, HIDDEN), F32, kind="ExternalOutput").ap()
    # DRAM scratch for the skew bounce: one [128, 2, 2, 4, 640] f8 block per
    # (batch-entry, head-pair).
    scr_t = nc.dram_tensor("skew_scratch",
                           (B // NCORES * HEADS // 2, 128, 2, 2, 4, WIN),
                           F8, kind="Internal")
    scr_d = scr_t.ap()
    SCR_PR = 128 * 2 * 2 * 4 * WIN  # elements per (bi, head-pair) block

    with tile.TileContext(nc) as tc, contextlib.ExitStack() as ctx:
        consts = ctx.enter_context(tc.tile_pool(name="consts", bufs=1))
        wpool = ctx.enter_context(tc.tile_pool(name="wpool", bufs=3))
        xio = ctx.enter_context(tc.tile_pool(name="xio", bufs=2))
        stat = ctx.enter_context(tc.tile_pool(name="stat", bufs=4))
        big = ctx.enter_context(tc.tile_pool(name="big", bufs=1))
        stagp = ctx.enter_context(tc.tile_pool(name="stagp", bufs=2))
        winp = ctx.enter_context(tc.tile_pool(name="winp", bufs=3))
        attp = ctx.enter_context(tc.tile_pool(name="attp", bufs=2))
        # PSUM: psA 3x1 bank (scores/fac/rb), psE 2x2 (expansion),
        # psC 1x1 (ctx) -> 8 banks total.
        psA = ctx.enter_context(tc.tile_pool(name="psA", bufs=3, space="PSUM"))
        psE = ctx.enter_context(tc.tile_pool(name="psE", bufs=2, space="PSUM"))
        psC = ctx.enter_context(tc.tile_pool(name="psC", bufs=1, space="PSUM"))

        # ---------- constants ----------
        ident16 = consts.tile([128, 128], F16)
        make_identity(nc, ident16)
        ident8 = consts.tile([128, 128], F8)
        make_identity(nc, ident8)
        eps_t = consts.tile([128, 1], F32)
        nc.vector.memset(eps_t, EPS)
        one64 = consts.tile([1, 64], F16)
        nc.vector.memset(one64, 1.0)
        gq_s = consts.tile([REL, 4, WIN], F8)
        gk_s = consts.tile([REL, 4, WIN], F8)
        mb_s = consts.tile([128, BLOC * 4], F32)
        if with_bias:
            bqk_s = consts.tile([1, 2 * HIDDEN], F16)
            bv_s = consts.tile([1, HIDDEN], F16)
            ones_s = consts.tile([1, TOK2], F16)
            nc.sync.dma_start(out=bqk_s, in_=bqk_d)
            nc.sync.dma_start(out=bv_s, in_=bv_d)
            nc.sync.dma_start(out=ones_s, in_=ones_d)
        if with_affine:
            g_s = consts.tile([128, HIDDEN], F32)
            b_s = consts.tile([128, HIDDEN], F32)
            nc.sync.dma_start(out=g_s, in_=g_d)
            nc.sync.dma_start(out=b_s, in_=b_d)

        def layernorm_stats(y):
            """-> (rstd, -mean*rstd) [128,1] tiles for the normalize apply."""
            st = stat.tile([128, 2, nc.vector.BN_STATS_DIM], F32, tag="st")
            mv = stat.tile([128, nc.vector.BN_AGGR_DIM], F32, tag="mv")
            yr = y.rearrange("p (s d) -> p s d", s=2)
            for s in range(2):
                nc.vector.bn_stats(out=st[:, s, :], in_=yr[:, s, :])
            nc.vector.bn_aggr(out=mv, in_=st)
            rstd = stat.tile([128, 1], F32, tag="rstd")
            nc.scalar.activation(out=rstd, in_=mv[:, 1:2], func=AF.Sqrt,
                                 bias=eps_t, scale=1.0)
            nc.vector.reciprocal(out=rstd, in_=rstd)
            nmr = stat.tile([128, 1], F32, tag="nmr")
            nc.vector.tensor_mul(nmr, mv[:, 0:1], rstd)
            nc.vector.tensor_scalar_mul(nmr, nmr, -1.0)
            return rstd, nmr

        # ---------- P1: LN1 + transpose (rel rows ride as extra tokens) ----
        hT = big.tile([128, NT, TOK2], F16, tag="hT")  # [feat, tok]
        hs3 = hs_d.rearrange("(n p) d -> n p d", p=128)

        def emit_ln_tile(tt):
            x = xio.tile([128, HIDDEN], F32, tag="xy")
            nc.sync.dma_start(out=x, in_=hs3[tt])
            rstd, nmr = layernorm_stats(x)
            h = xio.tile([128, HIDDEN], F16, tag="hyo")
            nc.vector.tensor_scalar(out=h, in0=x, scalar1=rstd, scalar2=nmr,
                                    op0=mybir.AluOpType.mult,
                                    op1=mybir.AluOpType.add)
            for fb in range(NT):
                ptr = psA.tile([128, 128], F16, tag="b1")
                nc.tensor.matmul(ptr, h[:, 128 * fb:128 * fb + 128], ident16,
                                 is_transpose=True)
                nc.vector.tensor_copy(out=hT[:, fb, 128 * tt:128 * tt + 128], in_=ptr)

        for tt in range(4):
            emit_ln_tile(tt)
        nc.scalar.dma_start(out=hT[:, :, NTOK:NTOK + 64],
                            in_=relT_d.rearrange("(n p) r -> p n r", p=128))
        nc.vector.memset(hT[:, :, TOK2 - 1:TOK2], 0.0)
        nc.scalar.dma_start(out=gq_s, in_=gq_d)
        nc.scalar.dma_start(out=gk_s, in_=gk_d)
        nc.scalar.dma_start(out=mb_s, in_=mb_d)

        # ---------- P1b: projections ----------
        qT = big.tile([128, NT, TOK2], F16, tag="qT")
        kT = big.tile([128, NT, TOK2], F16, tag="kT")
        vtm = big.tile([128, NT, HEADS, HEAD + 1], F16, tag="v")
        nc.vector.memset(vtm[:, :, :, HEAD:HEAD + 1], 1.0)
        wqk3 = wqkT_d

        # q/k (+qpos/kpos): feat-major out [2048 -> 16 M-tiles, 1088 tok]
        facsAll = big.tile([REL, BLOC * HEADS, 2, 512], F8, tag="facs")

        def emit_factors(hd):
            pf = slice(64 * (hd % 2), 64 * (hd % 2) + 64)
            mgh = hd // 2
            for bi in range(BLOC):
                toks = slice(512 * bi, 512 * bi + 512)
                bh = HEADS * bi + hd
                fq_ps = psA.tile([REL, 512], F32, tag="b1")
                nc.tensor.matmul(fq_ps, kT[pf, mgh, NTOK:NTOK + REL],
                                 qT[pf, mgh, toks])
                fk_ps = psA.tile([REL, 512], F32, tag="b1")
                nc.tensor.matmul(fk_ps, qT[pf, mgh, NTOK:NTOK + REL],
                                 kT[pf, mgh, toks])
                if bh % 2 == 0:
                    nc.scalar.copy(out=facsAll[:, bh, 0, :], in_=fq_ps)
                    nc.vector.tensor_copy(out=facsAll[:, bh, 1, :], in_=fk_ps)
                else:
                    nc.vector.tensor_copy(out=facsAll[:, bh, 0, :], in_=fq_ps)
                    nc.scalar.copy(out=facsAll[:, bh, 1, :], in_=fk_ps)

        NCH = [(0, 512), (512, 512), (1024, 64)]

        def emit_qk_pass(chunks, second):
            for mg in range(16):
                w_m = wpool.tile([128, 8, 128], F16, tag="wqk")
                nc.scalar.dma_start(out=w_m, in_=wqk3[mg])
                for ci in chunks:
                    n0, nw = NCH[ci]
                    ns = slice(n0, n0 + nw)
                    pq = psA.tile([128, 512], F32, tag="b1")
                    for k in range(8):
                        nc.tensor.matmul(pq[:, :nw], w_m[:, k, :], hT[:, k, ns],
                                         start=(k == 0),
                                         stop=(k == 7 and not with_bias))
                    if with_bias:
                        nc.tensor.matmul(pq[:, :nw],
                                         bqk_s[:, 128 * mg:128 * mg + 128],
                                         ones_s[:, ns], start=False, stop=True)
                    dst = qT if mg < 8 else kT
                    if (mg * 3 + ci) % 2 == 0:
                        nc.vector.tensor_copy(out=dst[:, mg % 8, ns],
                                              in_=pq[:, :nw])
                    else:
                        nc.scalar.copy(out=dst[:, mg % 8, ns], in_=pq[:, :nw])
                if second and mg >= 8:
                    emit_factors(2 * (mg - 8))
                    emit_factors(2 * (mg - 8) + 1)

        emit_qk_pass([0], False)        # tokens 0:512 (needs LN tiles 0-3)
        for tt in range(4, NT):
            emit_ln_tile(tt)
        emit_qk_pass([1, 2], True)      # tokens 512:1024 + rel, factors

        # v: token-major out [tok, feat]; wvT resident then woT reuses slot
        wv_s = big.tile([128, 8, HIDDEN], F16, tag="wvo")
        nc.sync.dma_start(out=wv_s, in_=wvT_d.rearrange("(n p) m -> p n m", p=128))
        for mt in range(NT):
            for nn_ in range(2):
                ns = slice(512 * nn_, 512 * nn_ + 512)
                pv = psA.tile([128, 512], F32, tag="b1")
                for k in range(8):
                    nc.tensor.matmul(pv, hT[:, k, 128 * mt:128 * mt + 128],
                                     wv_s[:, k, ns], start=(k == 0),
                                     stop=(k == 7 and not with_bias))
                if with_bias:
                    nc.tensor.matmul(pv, ones_s[:, 128 * mt:128 * mt + 128],
                                     bv_s[:, ns], start=False, stop=True)
                nc.vector.tensor_copy(
                    out=vtm[:, mt, 8 * nn_:8 * nn_ + 8, 0:HEAD],
                    in_=pv.rearrange("p (h d) -> p h d", d=HEAD))

        # wo load early (sync queue is idle; overlaps attention)
        wo_s = big.tile([128, 8, HIDDEN], F16, tag="wvo")  # reuse wv slot
        nc.sync.dma_start(out=wo_s, in_=woT_d.rearrange("(n p) m -> p n m", p=128))

        # ---------- P2: attention, software-pipelined by two heads ----------
        ctxT = big.tile([128, BLOC, NT, L], F16, tag="hT")  # reuse hT slot

        def emit_produce(bi, hp):
            """two heads' expansions -> staging -> one DRAM bounce -> windows."""
            pr = (HEADS * bi) // 2 + hp
            stag = stagp.tile([128, 2, 2, 4, WIN], F8, tag="stag")
            for hh in range(2):
                bh = HEADS * bi + 2 * hp + hh
                for s in range(2):
                    G = gq_s if s == 0 else gk_s
                    for t in range(4):
                        pe_ = psE.tile([128, 1024], F32, tag="exp")
                        nc.tensor.matmul(pe_[:, :512],
                                         facsAll[:, bh, s, 128 * t:128 * t + 128],
                                         G[:, t, :512])
                        nc.tensor.matmul(pe_[:, 512:WIN],
                                         facsAll[:, bh, s, 128 * t:128 * t + 128],
                                         G[:, t, 512:])
                        if (s * 4 + t) % 2 == 0:
                            nc.scalar.copy(out=stag[:, hh, s, t, :],
                                           in_=pe_[:, :WIN])
                        else:
                            nc.vector.tensor_copy(out=stag[:, hh, s, t, :],
                                                  in_=pe_[:, :WIN])
            nc.gpsimd.dma_start(out=scr_d[pr], in_=stag)
            win = winp.tile([128, 2, 2, 4, 512], F8, tag="win")
            # (hh, s) flatten to one dim: strides nest exactly (5120 = 2*2560)
            diag = AP(scr_t, pr * SCR_PR + 127,
                      [[2 * 2 * 4 * WIN - 1, 128], [4 * WIN, 4], [WIN, 4], [1, 512]])
            nc.gpsimd.dma_start(out=win.rearrange("p h s t j -> p (h s) t j"),
                                in_=diag)
            return win

        def emit_assembly(bi, hd, win):
            win = win[:, hd % 2]
            pf = slice(64 * (hd % 2), 64 * (hd % 2) + 64)
            mgh = hd // 2
            toks = slice(512 * bi, 512 * bi + 512)
            qTh = qT[pf, mgh, toks]
            kTh = kT[pf, mgh, toks]
            pctx = psC.tile([65, 512], F32, tag="ctx")
            for jt in range(4):
                js = slice(128 * jt, 128 * jt + 128)
                pst = psA.tile([128, 512], F32, tag="b1")
                nc.tensor.matmul(pst, kTh[:, js], qTh, start=True, stop=False)
                nc.tensor.matmul(pst, ident8, win[:, 1, jt, :],
                                 start=False, stop=False)
                for it in range(4):
                    nc.tensor.matmul(pst[:, 128 * it:128 * it + 128],
                                     win[:, 0, it, js], ident8,
                                     start=False, stop=(it == 3))
                P = attp.tile([128, 512], F16, tag="P")
                nc.scalar.activation(out=P, in_=pst, func=AF.Exp,
                                     bias=mb_s[:, 4 * bi + jt:4 * bi + jt + 1])
                nc.tensor.matmul(pctx, vtm[:, 4 * bi + jt, hd, :], P,
                                 start=(jt == 0), stop=(jt == 3))
            rsum = attp.tile([1, 512], F16, tag="rsum")
            with nc.allow_low_precision(reason="1/softmax-sum in f16 is ample"):
                nc.vector.reciprocal(out=rsum, in_=pctx[64:65, :])
            rb = psA.tile([64, 512], F32, tag="b1")
            nc.tensor.matmul(rb, one64, rsum)
            rb_s = attp.tile([64, 512], F16, tag="rbs")
            nc.vector.tensor_copy(out=rb_s, in_=rb)
            nc.vector.tensor_mul(ctxT[pf, bi, mgh, :], pctx[0:64, :], rb_s)

        out3 = out_d.rearrange("(n p) d -> n p d", p=128)

        def emit_p3(mt):
            bi, mtb = mt // 4, mt % 4
            y = xio.tile([128, HIDDEN], F32, tag="xy")
            for nn_ in range(2):
                ns = slice(512 * nn_, 512 * nn_ + 512)
                py = psA.tile([128, 512], F32, tag="b1")
                for k in range(8):
                    nc.tensor.matmul(
                        py, ctxT[:, bi, k, 128 * mtb:128 * mtb + 128],
                        wo_s[:, k, ns], start=(k == 0), stop=(k == 7))
                nc.scalar.copy(out=y[:, ns], in_=py)
            rstd, nmr = layernorm_stats(y)
            yo = xio.tile([128, HIDDEN], F32, tag="hyo")
            nc.vector.tensor_scalar(out=yo, in0=y, scalar1=rstd, scalar2=nmr,
                                    op0=mybir.AluOpType.mult,
                                    op1=mybir.AluOpType.add)
            if with_affine:
                nc.vector.tensor_mul(yo, yo, g_s)
                nc.vector.tensor_add(yo, yo, b_s)
            nc.sync.dma_start(out=out3[mt], in_=yo)

        order = [(bi, hp) for bi in range(BLOC) for hp in range(HEADS // 2)]
        pending = []
        for bi, hp in order:
            win = emit_produce(bi, hp)
            pending.append((bi, hp, win))
            if len(pending) > 2:
                pbi, php, pwin = pending.pop(0)
                emit_assembly(pbi, 2 * php, pwin)
                emit_assembly(pbi, 2 * php + 1, pwin)
        for pbi, php, pwin in pending:
            emit_assembly(pbi, 2 * php, pwin)
            emit_assembly(pbi, 2 * php + 1, pwin)

        for mt in range(NT):
            emit_p3(mt)

    nc.compile()
    return nc


_CACHE = {}


def _get_nc(with_bias, with_affine):
    key = (with_bias, with_affine)
    if key not in _CACHE:
        _CACHE[key] = _build(with_bias, with_affine)
    return _CACHE[key]


def _host_prep(inputs):
    hs = np.ascontiguousarray(np.asarray(inputs["hidden_states"], np.float32))
    mask = np.asarray(inputs["attention_mask"])
    rel = np.asarray(inputs["relative_embedding"], np.float32)
    wqk = np.asarray(inputs["wqk"], np.float32)
    bqk = np.asarray(inputs["bqk"], np.float32)
    wv = np.asarray(inputs["wv"], np.float32)
    bv = np.asarray(inputs["bv"], np.float32)
    wo = np.asarray(inputs["wo"], np.float32)
    bo = np.asarray(inputs["bo"], np.float32)
    ln_g = np.asarray(inputs["ln_g"], np.float32)
    ln_b = np.asarray(inputs["ln_b"], np.float32)

    assert np.all(bo == 0.0), "kernel relies on bo == 0 (softmax-in-LN cancellation)"

    with_bias = bool(np.any(bqk != 0) or np.any(bv != 0))
    with_affine = bool(np.any(ln_g != 1) or np.any(ln_b != 0))

    wqkT = np.ascontiguousarray(wqk.T).astype(np.float64)
    wqkT[:, :HIDDEN] *= SCALE
    wqkT = wqkT.astype(np.float16)
    # [mg, p, k, j] = wqkT[128k + p, 128mg + j]: contiguous per-mg weight tiles
    wqkT = np.ascontiguousarray(
        wqkT.reshape(8, 128, 16, 128).transpose(2, 1, 0, 3))
    bqk2 = bqk.astype(np.float64)
    bqk2[:HIDDEN] *= SCALE
    bqk2 = bqk2.astype(np.float16)
    wvT = np.ascontiguousarray(wv.T).astype(np.float16)
    woT = np.ascontiguousarray(wo.T).astype(np.float16)
    relT = np.zeros((HIDDEN, 64), np.float16)
    relT[:, :REL] = rel.T
    Gq, Gk = _make_G()

    shared = {"wqkT": wqkT, "wvT": wvT, "woT": woT, "relT": relT,
              "Gq": Gq, "Gk": Gk}
    if with_bias:
        shared["bqk2"] = bqk2.reshape(1, -1)
        shared["bv2"] = bv.astype(np.float16).reshape(1, -1)
        shared["ones_row"] = np.ones((1, TOK2), np.float16)
    if with_affine:
        shared["g_bcast"] = np.ascontiguousarray(
            np.broadcast_to(ln_g, (128, HIDDEN)))
        shared["b_bcast"] = np.ascontiguousarray(
            np.broadcast_to(ln_b, (128, HIDDEN)))

    in_maps = []
    for c in range(NCORES):
        m = dict(shared)
        hs_c = hs[:, 2 * c:2 * c + 2, :]
        m["hs_tok"] = np.ascontiguousarray(
            hs_c.transpose(1, 0, 2).reshape(NTOK, HIDDEN))
        mb = np.zeros((128, BLOC * 4), np.float32)
        for bi in range(BLOC):
            mrow = np.asarray(mask[2 * c + bi, 0, 0, :])
            for t in range(4):
                mb[:, 4 * bi + t] = np.where(mrow[128 * t:128 * t + 128], -1e9, 0.0)
        m["maskbias"] = mb
        in_maps.append(m)
    return in_maps, with_bias, with_affine


def kernel(**inputs):
    in_maps, with_bias, with_affine = _host_prep(inputs)
    nc = _get_nc(with_bias, with_affine)
    res = bass_utils.run_bass_kernel_spmd(nc, in_maps, core_ids=list(range(NCORES)))
    out = np.zeros((L, B, HIDDEN), np.float32)
    for c in range(NCORES):
        y = res.results[c]["out_y"]  # (NTOK, HIDDEN) token-major
        for bi in range(BLOC):
            out[:, 2 * c + bi, :] = y[512 * bi:512 * bi + 512, :]
    return out
